# revision 1
# baseline (speedup 1.0000x reference)
"""Bidirectional GATConv + fusion + BatchNorm + ReLU on 8 Trainium2 cores.

Strategy: nodes sharded 8 ways by aggregation target. Each core:
  1. projects x -> h_f/h_b + attention logits (replicated compute, bf16 PE),
     writing gather tables [h | a_s] (768B rows) and a local a_d table.
  2. walks its incident edges (dst-sorted, host-partitioned) in 128-edge
     chunks: dma_gather of source rows, softmax weights via exp(lrelu),
     scatter-add into PSUM via one-hot matmul (lhsT = S, built on DVE by
     iota==dstpos compare).
  3. fuses [fwd|bwd] @ W_fuse, computes BN stats, AllReduces them (4KB),
     normalizes + ReLU, writes its 1/8 output shard.
Biases provably cancel through BatchNorm and are dropped.
"""
import sys

sys.path.insert(0, "/opt/trn_rl_repo")

import numpy as np
import ml_dtypes

import concourse.bass as bass
import concourse.bacc as bacc
import concourse.mybir as mybir
from concourse import tile
from concourse import library_config
from concourse.bass_utils import run_bass_kernel_spmd

bf16 = mybir.dt.bfloat16
f32 = mybir.dt.float32
i16 = mybir.dt.int16
Alu = mybir.AluOpType
Act = mybir.ActivationFunctionType

NCORES = 8
USE_CC = __import__("os").environ.get("NO_CC", "0") != "1"
NO_FUSE = __import__("os").environ.get("NO_FUSE", "0") == "1"
NO_EDGE = __import__("os").environ.get("NO_EDGE", "0") == "1"
KB = 4          # dst blocks per gather supergroup
NEG_SLOPE = 0.2
BN_EPS = 1e-5
DUMMY_AS = -60.0


def _derive(n_nodes):
    npc = n_nodes // NCORES
    nb = (npc + 127) // 128
    half = ((n_nodes // 2) // 128) * 128
    trows_lo = half + 64           # dummy row at index `half`
    trows_hi = (n_nodes - half) + NCORES * 16 + 64  # covers proj padding rows
    return npc, nb, half, trows_lo, trows_hi


def _pack_idx(arr):
    """int16 [n] (n%16==0) -> [128, n/16] wrapped in 16 partitions, replicated per Q7 core."""
    a = arr.reshape(-1, 16).T
    return np.tile(a, (8, 1)).astype(np.int16)


def _prep_edges(gidx, anode, n_nodes):
    """Host edge partitioning for one direction.

    gidx: gather-side node per edge; anode: aggregation node per edge.
    Returns per-core chunk tensors with a uniform (CLO, CHI) template.
    """
    npc, nb, half, _, _ = _derive(n_nodes)
    n_all = ((n_nodes + 127) // 128) * 128
    dlo, dhi = half + 16, (n_all - half) + 16
    core = anode // npc
    local = anode - core * npc
    block = local // 128
    dstpos = local % 128
    hi = (gidx >= half).astype(np.int64)
    lidx = gidx - hi * half

    counts = np.zeros((NCORES, nb, 2), dtype=np.int64)
    np.add.at(counts, (core, block, hi), 1)
    nchunks = -(-counts // 128)  # ceil
    clo = int(nchunks[:, :, 0].max())
    chi = int(nchunks[:, :, 1].max())

    order = np.lexsort((lidx, hi, block, core))
    g_s, blk_s, hi_s, lidx_s, dp_s = (
        core[order], block[order], hi[order], lidx[order], dstpos[order])

    cmax = max(clo, chi)
    g1 = np.empty((NCORES, nb, 2, cmax * 128), dtype=np.int16)
    g1[:, :, 0, :] = dlo
    g1[:, :, 1, :] = dhi
    g2 = np.zeros((NCORES, nb, 2, cmax * 128), dtype=np.int16)
    dp = np.full((NCORES, nb, 2, cmax * 128), 200.0, dtype=ml_dtypes.bfloat16)

    # fill positions within each (core, block, hi) group
    flat_grp = (g_s * nb + blk_s) * 2 + hi_s
    # positions via cumcount
    idx_sorted = np.argsort(flat_grp, kind="stable")
    fg = flat_grp[idx_sorted]
    pos = np.arange(len(fg)) - np.concatenate(([0], np.cumsum(np.bincount(fg, minlength=NCORES*nb*2))))[fg]
    c_, b_, h_ = fg // (nb * 2), (fg // 2) % nb, fg % 2
    g1[c_, b_, h_, pos] = lidx_s[idx_sorted].astype(np.int16)
    g2[c_, b_, h_, pos] = (blk_s[idx_sorted] * 128 + dp_s[idx_sorted]).astype(np.int16)
    g2[:, :, :, :][g2 < 0] = 0
    # pad g2 entries point at the block's first row
    padmask = np.ones((NCORES, nb, 2, cmax * 128), dtype=bool)
    padmask[c_, b_, h_, pos] = False
    bb = np.broadcast_to(np.arange(nb)[None, :, None, None] * 128,
                         (NCORES, nb, 2, cmax * 128))
    g2[padmask] = bb[padmask].astype(np.int16)
    dp[c_, b_, h_, pos] = dp_s[idx_sorted].astype(ml_dtypes.bfloat16)

    # trim halves to their own chunk counts
    g1lo, g1hi = g1[:, :, 0, :clo * 128], g1[:, :, 1, :chi * 128]
    g2lo, g2hi = g2[:, :, 0, :clo * 128], g2[:, :, 1, :chi * 128]
    dplo, dphi = dp[:, :, 0, :clo * 128], dp[:, :, 1, :chi * 128]

    # gather idx streams: per supergroup: lo idxs then hi idxs (block-major)
    g1_streams, g2_streams = [], []
    for c in range(NCORES):
        p1, p2 = [], []
        for bs in range(0, nb, KB):
            be = min(bs + KB, nb)
            p1 += [_pack_idx(g1lo[c, bs:be].ravel()), _pack_idx(g1hi[c, bs:be].ravel())]
            p2 += [_pack_idx(g2lo[c, bs:be].ravel()), _pack_idx(g2hi[c, bs:be].ravel())]
        g1_streams.append(np.concatenate(p1, axis=1))
        g2_streams.append(np.concatenate(p2, axis=1))
    # dstpos stream [128, nb*(clo+chi)] block-major, lo chunks then hi chunks
    dpl = dplo.reshape(NCORES, nb, clo, 128).transpose(0, 3, 1, 2).reshape(NCORES, 128, nb * clo)
    dph = dphi.reshape(NCORES, nb, chi, 128).transpose(0, 3, 1, 2).reshape(NCORES, 128, nb * chi)
    dp_stream = np.concatenate([dpl, dph], axis=2)
    return clo, chi, np.stack(g1_streams), np.stack(g2_streams), np.ascontiguousarray(dp_stream)


def _build_program(n_nodes, clo_f, chi_f, clo_b, chi_b):
    npc, nb, half, trows_lo, trows_hi = _derive(n_nodes)
    nbr = npc - (nb - 1) * 128            # rows in last block
    npad = nb * 128
    n_all = ((n_nodes + 127) // 128) * 128
    npb = n_all // 128                     # projection node blocks
    dummy_lo = half + 16
    dummy_hi = (n_all - half) + 16

    nc = bacc.Bacc(None, target_bir_lowering=False)
    inp = lambda name, shape, dt: nc.declare_dram_parameter(name, shape, dt, isOutput=False)
    xT = inp("xT", [512, n_all], bf16)
    xTo = inp("xTo", [512, npad], bf16)
    wall = inp("wall", [512, 528], bf16)
    wfuse = inp("wfuse", [512, 512], bf16)
    drow = inp("drow", [1, 384], bf16)
    iota_in = inp("iota", [128, 128], bf16)
    ident = inp("ident", [128, 128], f32)
    bnpg = inp("bnpg", [32, 128], f32)
    bnpb = inp("bnpb", [32, 128], f32)
    streams = {}
    for d, (clo, chi) in (("f", (clo_f, chi_f)), ("b", (clo_b, chi_b))):
        tc_d = nb * (clo + chi)
        streams["g1" + d] = inp("g1" + d, [128, tc_d * 8], i16)
        streams["g2" + d] = inp("g2" + d, [128, tc_d * 8], i16)
        streams["dp" + d] = inp("dp" + d, [128, tc_d], f32)
    out_d = nc.declare_dram_parameter("out", [npc, 512], f32, isOutput=True)

    tabs = {d: [nc.dram_tensor(f"tab{d}{h}", [tr, 384], bf16)
                for h, tr in (("lo", trows_lo), ("hi", trows_hi))] for d in "fb"}
    adtab = nc.dram_tensor("adtab", [npad, 128], bf16)
    combined = nc.dram_tensor("combined", [npad, 512], bf16)
    ccin = nc.dram_tensor("ccin", [128, 64], f32)
    ccout = nc.dram_tensor("ccout", [128, 64], f32, addr_space="Shared")
    abtmp = nc.dram_tensor("abtmp", [8, 128], f32)

    hblocks = half // 128

    with tile.TileContext(nc) as tc:
        with (
            tc.tile_pool(name="const", bufs=1) as cpool,
        ):
            nc.gpsimd.load_library(library_config.mlp)
            wall_sb = cpool.tile([128, 4, 528], bf16)
            for k in range(4):
                nc.sync.dma_start(wall_sb[:, k, :], wall[k * 128:(k + 1) * 128, :])
            iota_sb = cpool.tile([128, 128], bf16)
            nc.sync.dma_start(iota_sb[:], iota_in[:])
            # zero-fill unwritten table tail rows, then dummy rows
            zt = cpool.tile([128, 384], bf16)
            nc.gpsimd.memset(zt[:], 0.0)
            for b in range(npad // 128):
                nc.sync.dma_start(adtab[b * 128:(b + 1) * 128, :], zt[:, 0:128])
            for d in "fb":
                r = half
                while r < trows_lo:
                    n = min(128, trows_lo - r)
                    nc.sync.dma_start(tabs[d][0][r:r + n, :], zt[0:n, :])
                    r += n
                r = n_all - half
                while r < trows_hi:
                    n = min(128, trows_hi - r)
                    nc.sync.dma_start(tabs[d][1][r:r + n, :], zt[0:n, :])
                    r += n
                nc.sync.dma_start(tabs[d][0][dummy_lo:dummy_lo + 1, :], drow[:])
                nc.sync.dma_start(tabs[d][1][dummy_hi:dummy_hi + 1, :], drow[:])

            # ---------------- projection ----------------
            with (tc.tile_pool(name="proj", bufs=3) as pj,
                  tc.tile_pool(name="pspj", bufs=2, space="PSUM") as pp):
                for nbk in range(npb):
                    xt = pj.tile([128, 4, 128], bf16, tag="xt")
                    nc.sync.dma_start(
                        xt[:], xT[:, nbk * 128:(nbk + 1) * 128]
                        .rearrange("(k p) n -> p k n", p=128))
                    ps = {d: pp.tile([128, 260], f32, tag="pj" + d, name=f"ps{d}_{nbk}")
                          for d in "fb"}
                    for k in range(4):
                        nc.tensor.matmul(ps["f"][:], xt[:, k, :], wall_sb[:, k, 0:260],
                                         start=(k == 0), stop=(k == 3))
                    for k in range(4):
                        nc.tensor.matmul(ps["b"][:], xt[:, k, :], wall_sb[:, k, 260:520],
                                         start=(k == 0), stop=(k == 3))
                    for d, eng in (("f", nc.scalar), ("b", nc.vector)):
                        st = pj.tile([128, 384], bf16, tag="st" + d)
                        if d == "f":
                            eng.activation(st[:, 0:260], ps[d][:], Act.Copy)
                        else:
                            eng.tensor_copy(st[:, 0:260], ps[d][:])
                        nc.gpsimd.memset(st[:, 260:384], 0.0)
                        if nbk < hblocks:
                            dst = tabs[d][0][nbk * 128:(nbk + 1) * 128, :]
                        else:
                            r0 = (nbk - hblocks) * 128
                            dst = tabs[d][1][r0:r0 + 128, :]
                        nc.sync.dma_start(dst, st[:])
                # local a_d table
                ad_stage = pj.tile([128, nb, 8], bf16, tag="ad")
                for b in range(nb):
                    xo = pj.tile([128, 4, 128], bf16, tag="xo")
                    nc.sync.dma_start(
                        xo[:], xTo[:, b * 128:(b + 1) * 128]
                        .rearrange("(k p) n -> p k n", p=128))
                    pa = pp.tile([128, 8], f32, tag="pa", bufs=1)
                    for k in range(4):
                        nc.tensor.matmul(pa[:], xo[:, k, :], wall_sb[:, k, 520:528],
                                         start=(k == 0), stop=(k == 3))
                    nc.vector.tensor_copy(ad_stage[:, b, :], pa[:])
                nc.sync.dma_start(
                    adtab.rearrange("(b p) c -> p b c", p=128)[:, :, 0:8], ad_stage[:])

            # ---------------- edge passes ----------------
            edirs = () if NO_EDGE else (("f", clo_f, chi_f, 0), ("b", clo_b, chi_b, 256))
            for d, clo, chi, dcol in edirs:
                tc_d = nb * (clo + chi)
                with tc.tile_pool(name="edge" + d, bufs=1) as ep:
                    dp_sb = ep.tile([128, tc_d], f32)
                    nc.sync.dma_start(dp_sb[:], streams["dp" + d][:])
                    g1_sb = ep.tile([128, tc_d * 8], i16)
                    nc.sync.dma_start(g1_sb[:], streams["g1" + d][:])
                    g2_sb = ep.tile([128, tc_d * 8], i16)
                    nc.sync.dma_start(g2_sb[:], streams["g2" + d][:])
                    with (tc.tile_pool(name="ew" + d, bufs=2) as ew,
                      tc.tile_pool(name="psed" + d, bufs=4, space="PSUM") as pp):
                        c1 = c2 = 0  # stream column cursors
                        for bs in range(0, nb, KB):
                            kbs = min(KB, nb - bs)
                            tiles = {}
                            for hname, cc, tabx in (("lo", clo, tabs[d][0]),
                                                    ("hi", chi, tabs[d][1])):
                                ni = kbs * cc * 128
                                gt = ew.tile([128, kbs * cc, 384], bf16, tag="g1t" + hname)
                                nc.gpsimd.dma_gather(
                                    gt[:], tabx[:], g1_sb[:, c1:c1 + ni // 16],
                                    num_idxs=ni, num_idxs_reg=ni, elem_size=384, single_packet=False)
                                c1 += ni // 16
                                at = ew.tile([128, kbs * cc, 128], bf16, tag="g2t" + hname)
                                nc.gpsimd.dma_gather(
                                    at[:], adtab[:], g2_sb[:, c2:c2 + ni // 16],
                                    num_idxs=ni, num_idxs_reg=ni, elem_size=128, single_packet=False)
                                c2 += ni // 16
                                tiles[hname] = (gt, at, cc)
                            for j in range(kbs):
                                b = bs + j
                                pb = pp.tile([128, 260], f32, tag="pb")
                                first = True
                                for hname in ("lo", "hi"):
                                    gt, at, cc = tiles[hname]
                                    if cc == 0:
                                        continue
                                    hofs = 0 if hname == "lo" else nb * clo
                                    tc0 = hofs + b * cc
                                    adofs = dcol // 256 * 4  # f: cols 0:4, b: 4:8
                                    et = ew.tile([128, cc, 4], bf16, tag="et")
                                    for k in range(cc):
                                        nc.vector.tensor_tensor(
                                            et[:, k, :], gt[:, j * cc + k, 256:260],
                                            at[:, j * cc + k, adofs:adofs + 4], Alu.add)
                                    lt2 = ew.tile([128, cc, 4], bf16, tag="lt2")
                                    nc.vector.tensor_scalar_mul(
                                        lt2[:].rearrange("p c h -> p (c h)"),
                                        et[:].rearrange("p c h -> p (c h)"), NEG_SLOPE)
                                    lt = ew.tile([128, cc, 4], f32, tag="lt")
                                    nc.vector.tensor_tensor(
                                        lt[:].rearrange("p c h -> p (c h)"),
                                        et[:].rearrange("p c h -> p (c h)"),
                                        lt2[:].rearrange("p c h -> p (c h)"), Alu.max)
                                    ext = ew.tile([128, cc, 4], f32, tag="ext")
                                    nc.scalar.activation(ext[:], lt[:], Act.Exp)
                                    mt = ew.tile([128, cc, 260], bf16, tag="mt")
                                    for k in range(cc):
                                        for h in range(2):
                                            nc.vector.tensor_scalar(
                                                mt[:, k, h * 64:(h + 1) * 64],
                                                gt[:, j * cc + k, h * 64:(h + 1) * 64],
                                                ext[:, k, h:h + 1], None, op0=Alu.mult)
                                        for h in range(2, 4):
                                            nc.scalar.activation(
                                                mt[:, k, h * 64:(h + 1) * 64],
                                                gt[:, j * cc + k, h * 64:(h + 1) * 64],
                                                Act.Copy, scale=ext[:, k, h:h + 1])
                                    for k in range(cc):
                                        nc.scalar.activation(mt[:, k, 256:260],
                                                             ext[:, k, :], Act.Copy)
                                    st = ew.tile([128, cc, 128], bf16, tag="st")
                                    for k in range(cc):
                                        nc.vector.tensor_scalar(
                                            st[:, k, :], iota_sb[:],
                                            dp_sb[:, tc0 + k:tc0 + k + 1], None,
                                            op0=Alu.is_equal)
                                    for k in range(cc):
                                        last = (hname == "hi" or chi == 0) and k == cc - 1
                                        nc.tensor.matmul(pb[:], st[:, k, :], mt[:, k, :],
                                                         start=first, stop=last)
                                        first = False
                                dn = ew.tile([128, 4], f32, tag="dn")
                                nc.vector.tensor_scalar_add(dn[:], pb[:, 256:260], 1e-16)
                                rc = ew.tile([128, 4], f32, tag="rc")
                                nc.vector.reciprocal(rc[:], dn[:])
                                ob = ew.tile([128, 256], bf16, tag="ob")
                                for h in range(4):
                                    nc.vector.tensor_scalar(
                                        ob[:, h * 64:(h + 1) * 64], pb[:, h * 64:(h + 1) * 64],
                                        rc[:, h:h + 1], None, op0=Alu.mult)
                                nc.sync.dma_start(
                                    combined[b * 128:(b + 1) * 128, dcol:dcol + 256], ob[:])

            # ---------------- fusion + BN ----------------
            if NO_FUSE:
                with tc.tile_pool(name="nf", bufs=1) as nf:
                    z = nf.tile([128, 512], f32)
                    nc.vector.memset(z[:], 0.0)
                    for b in range(nb):
                        rows = min(128, npc - b * 128)
                        nc.sync.dma_start(out_d[b * 128:b * 128 + rows, :], z[0:rows, :])
            if not NO_FUSE:
             with (tc.tile_pool(name="fuse", bufs=1) as fp,
                  tc.tile_pool(name="psfu", bufs=1, space="PSUM") as pp):
                wf_sb = fp.tile([128, 4, 512], bf16)
                for k in range(4):
                    nc.sync.dma_start(wf_sb[:, k, :], wfuse[k * 128:(k + 1) * 128, :])
                combT = [fp.tile([128, npad], bf16, tag=f"ct{k}", name=f"ct{k}")
                         for k in range(4)]
                for k in range(4):
                    nc.sync.dma_start_transpose(combT[k][:], combined[:, k * 128:(k + 1) * 128])
                acc = fp.tile([128, 512], f32)
                acc2 = fp.tile([128, 512], f32)
                nc.vector.memset(acc[:], 0.0)
                nc.vector.memset(acc2[:], 0.0)
                fused = fp.tile([128, nb, 512], bf16)
                with tc.tile_pool(name="fw", bufs=3) as fw:
                    for b in range(nb):
                        pf = pp.tile([128, 512], f32, tag="pf", bufs=2)
                        for k in range(4):
                            nc.tensor.matmul(pf[:], combT[k][:, b * 128:(b + 1) * 128],
                                             wf_sb[:, k, :], start=(k == 0), stop=(k == 3))
                        nc.scalar.activation(fused[:, b, :], pf[:], Act.Copy)
                        pfs = fw.tile([128, 512], f32, tag="pfs")
                        nc.vector.tensor_copy(pfs[:], pf[:])
                        sq = fw.tile([128, 512], f32, tag="sq")
                        nc.vector.tensor_tensor(sq[:], pfs[:], pfs[:], Alu.mult)
                        nc.vector.tensor_tensor(acc[:], acc[:], pfs[:], Alu.add)
                        nc.vector.tensor_tensor(acc2[:], acc2[:], sq[:], Alu.add)
                    # partition-reduce stats via ones matmul
                    ones = fw.tile([128, 1], f32, tag="ones")
                    nc.vector.memset(ones[:], 1.0)
                    stat = fw.tile([128, 64], f32, tag="stat")
                    nc.vector.memset(stat[:], 0.0)
                    for k in range(4):
                        psk = pp.tile([128, 1], f32, tag="psk")
                        nc.tensor.matmul(psk[:], acc[:, k * 128:(k + 1) * 128], ones[:])
                        nc.vector.tensor_copy(stat[:, k:k + 1], psk[:])
                        psk2 = pp.tile([128, 1], f32, tag="psk")
                        nc.tensor.matmul(psk2[:], acc2[:, k * 128:(k + 1) * 128], ones[:])
                        nc.vector.tensor_copy(stat[:, 32 + k:33 + k], psk2[:])
                    nc.sync.dma_start(ccin[:], stat[:])
                    sg_sb = fw.tile([128, 64], f32, tag="sg")
                    if USE_CC:
                        nc.gpsimd.collective_compute(
                            "AllReduce", Alu.add, replica_groups=[list(range(NCORES))],
                            ins=[ccin[:]], outs=[ccout[:]])
                        nc.sync.dma_start(sg_sb[:], ccout[:])
                    else:
                        nc.sync.dma_start(sg_sb[:], ccin[:])
                    # transpose stats to row layout
                    id_sb = fw.tile([128, 128], f32, tag="id")
                    nc.sync.dma_start(id_sb[:], ident[:])
                    pt1 = pp.tile([32, 128], f32, tag="pt1")
                    nc.tensor.transpose(pt1[:], sg_sb[:, 0:32], id_sb[:])
                    pt2 = pp.tile([32, 128], f32, tag="pt2")
                    nc.tensor.transpose(pt2[:], sg_sb[:, 32:64], id_sb[:])
                    gam_t = fw.tile([32, 128], f32, tag="gam")
                    nc.sync.dma_start(gam_t[:], bnpg[:])
                    bet_t = fw.tile([32, 128], f32, tag="bet")
                    nc.sync.dma_start(bet_t[:], bnpb[:])
                    m = fw.tile([32, 128], f32, tag="m")
                    nc.vector.tensor_scalar_mul(m[:], pt1[:], 1.0 / n_nodes)
                    e2 = fw.tile([32, 128], f32, tag="e2")
                    nc.vector.tensor_scalar_mul(e2[:], pt2[:], 1.0 / n_nodes)
                    msq = fw.tile([32, 128], f32, tag="msq")
                    nc.vector.tensor_tensor(msq[:], m[:], m[:], Alu.mult)
                    var = fw.tile([32, 128], f32, tag="var")
                    nc.vector.tensor_tensor(var[:], e2[:], msq[:], Alu.subtract)
                    nc.vector.tensor_scalar_add(var[:], var[:], BN_EPS)
                    sd = fw.tile([32, 128], f32, tag="sd")
                    nc.scalar.activation(sd[:], var[:], Act.Sqrt)
                    rs = fw.tile([32, 128], f32, tag="rs")
                    nc.vector.reciprocal(rs[:], sd[:])
                    A = fw.tile([32, 128], f32, tag="A")
                    nc.vector.tensor_tensor(A[:], rs[:], gam_t[:], Alu.mult)
                    mA = fw.tile([32, 128], f32, tag="mA")
                    nc.vector.tensor_tensor(mA[:], m[:], A[:], Alu.mult)
                    B = fw.tile([32, 128], f32, tag="B")
                    nc.vector.tensor_tensor(B[:], bet_t[:], mA[:], Alu.subtract)
                    nc.sync.dma_start(abtmp[0:4, :], A[0:4, :])
                    nc.sync.dma_start(abtmp[4:8, :], B[0:4, :])
                    ab_sb = fw.tile([1, 1024], f32, tag="ab")
                    nc.sync.dma_start(ab_sb[:], abtmp.rearrange("a b -> (a b)")[None, :])
                    ones1 = fw.tile([1, 128], f32, tag="o1")
                    nc.vector.memset(ones1[:], 1.0)
                    pA = pp.tile([128, 512], f32, tag="pA")
                    nc.tensor.matmul(pA[:], ones1[:], ab_sb[:, 0:512])
                    pB = pp.tile([128, 512], f32, tag="pB")
                    nc.tensor.matmul(pB[:], ones1[:], ab_sb[:, 512:1024])
                    for b in range(nb):
                        t0 = fw.tile([128, 512], f32, tag="t0")
                        nc.scalar.activation(t0[:], fused[:, b, :], Act.Copy)
                        t1 = fw.tile([128, 512], f32, tag="t1")
                        nc.vector.tensor_tensor(t1[:], t0[:], pA[:], Alu.mult)
                        nc.vector.tensor_tensor(t1[:], t1[:], pB[:], Alu.add)
                        nc.vector.tensor_scalar_max(t1[:], t1[:], 0.0)
                        rows = min(128, npc - b * 128)
                        nc.sync.dma_start(out_d[b * 128:b * 128 + rows, :], t1[0:rows, :])
    nc.compile()
    return nc


def kernel(**inputs):
    x = np.asarray(inputs["x"], dtype=np.float32)
    ei = np.asarray(inputs["edge_index"])
    n_nodes, D = x.shape
    npc, nb, half, trows_lo, trows_hi = _derive(n_nodes)
    n_all = ((n_nodes + 127) // 128) * 128
    npad = nb * 128

    def g(name):
        return np.asarray(inputs[name], dtype=np.float32)

    W_f, W_b = g("W_f"), g("W_b")
    asf, adf = g("att_src_f"), g("att_dst_f")
    asb, adb = g("att_src_b"), g("att_dst_b")
    W_fuse = g("W_fuse")
    gamma, beta = g("bn_gamma"), g("bn_beta")

    wall = np.zeros((512, 528), dtype=np.float32)
    wall[:, 0:256] = W_f.reshape(512, 256)
    wall[:, 256:260] = np.einsum("dhc,hc->dh", W_f, asf)
    wall[:, 260:516] = W_b.reshape(512, 256)
    wall[:, 516:520] = np.einsum("dhc,hc->dh", W_b, asb)
    wall[:, 520:524] = np.einsum("dhc,hc->dh", W_f, adf)
    wall[:, 524:528] = np.einsum("dhc,hc->dh", W_b, adb)

    xT = np.zeros((512, n_all), dtype=ml_dtypes.bfloat16)
    xT[:, :n_nodes] = x.T
    drow = np.zeros((1, 384), dtype=ml_dtypes.bfloat16)
    drow[0, 256:260] = DUMMY_AS
    iota = np.broadcast_to(np.arange(128), (128, 128)).astype(ml_dtypes.bfloat16)
    ident = np.eye(128, dtype=np.float32)
    bnpg = np.zeros((32, 128), dtype=np.float32); bnpg[0:4] = gamma.reshape(4, 128)
    bnpb = np.zeros((32, 128), dtype=np.float32); bnpb[0:4] = beta.reshape(4, 128)

    src, dst = ei[0].astype(np.int64), ei[1].astype(np.int64)
    clo_f, chi_f, g1f, g2f, dpf = _prep_edges(src, dst, n_nodes)
    clo_b, chi_b, g1b, g2b, dpb = _prep_edges(dst, src, n_nodes)

    nc = _build_program(n_nodes, clo_f, chi_f, clo_b, chi_b)

    in_maps = []
    for c in range(NCORES):
        xTo = np.zeros((512, npad), dtype=ml_dtypes.bfloat16)
        xTo[:, :npc] = x.T[:, c * npc:(c + 1) * npc]
        in_maps.append({
            "xT": xT, "xTo": xTo,
            "wall": wall.astype(ml_dtypes.bfloat16),
            "wfuse": W_fuse.astype(ml_dtypes.bfloat16),
            "drow": drow, "iota": iota, "ident": ident, "bnpg": bnpg, "bnpb": bnpb,
            "g1f": g1f[c], "g2f": g2f[c], "dpf": dpf[c].astype(np.float32),
            "g1b": g1b[c], "g2b": g2b[c], "dpb": dpb[c].astype(np.float32),
        })
    kernel.last_nc = nc
    res = run_bass_kernel_spmd(nc, in_maps, list(range(NCORES)))
    out = np.concatenate([np.asarray(res.results[c]["out"]) for c in range(NCORES)], axis=0)
    return out[:n_nodes].astype(np.float32)


if __name__ == "__main__":
    pass



# revision 12
# speedup vs baseline: 1.8700x; 1.8700x over previous
"""Bidirectional GATConv + fusion + BatchNorm + ReLU on 8 Trainium2 cores.

v2 design (cost-model driven):
  - DMA descriptor-time is the hard floor (all transfers serialize on the
    DMA_ENGINES device). Tables shrunk to 520B rows (260 bf16 cols,
    (c,h)-interleaved), a_d table to 32B rows, xT loaded with 2KB
    descriptors, no `combined` DRAM round-trip.
  - Edge inner loop: one batched DVE tensor_tensor for the alpha-weighting
    (ext broadcast rides a non-last stride-0 dim so 2x bf16 packing holds),
    one tensor_scalar one-hot per chunk, exp writes straight into the
    message tile's denominator columns so one 260-row matmul per chunk
    scatters messages + denominators.
  - Fusion per dst-block via PE transposes (no DRAM transpose), BN stats
    via ones-matmul PSUM accumulation, 4KB AllReduce, normalize+ReLU
    split across DVE/Pool.
Biases provably cancel through BatchNorm and are dropped.
"""
import sys

sys.path.insert(0, "/opt/trn_rl_repo")

import numpy as np
import ml_dtypes

import concourse.bass as bass
import concourse.bacc as bacc
import concourse.mybir as mybir
from concourse import tile
from concourse import library_config
from concourse.bass_utils import run_bass_kernel_spmd

bf16 = mybir.dt.bfloat16
f32 = mybir.dt.float32
i16 = mybir.dt.int16
Alu = mybir.AluOpType
Act = mybir.ActivationFunctionType

NCORES = 8
USE_CC = __import__("os").environ.get("NO_CC", "0") != "1"
KB = 2          # dst blocks per gather supergroup
NEG_SLOPE = 0.2
BN_EPS = 1e-5
DUMMY_AS = -60.0
TROW = 260      # written cols: 256 (c,h)-interleaved h + 4 a_s
TSTRIDE = 384   # physical table row stride (768B, gather elem must be 256B-mult)
XW = 1024       # xT load column batch (2KB descriptors)


def _derive(n_nodes):
    npc = n_nodes // NCORES
    nb = (npc + 127) // 128
    half = ((n_nodes // 2) // 128) * 128
    trows_lo = half + 64            # dummy row at half+16
    trows_hi = (n_nodes - half) + NCORES * 16 + 64
    return npc, nb, half, trows_lo, trows_hi


def _pack_idx(arr):
    """int16 [n] (n%16==0) -> [128, n/16] wrapped in 16 partitions, replicated per Q7 core."""
    a = arr.reshape(-1, 16).T
    return np.tile(a, (8, 1)).astype(np.int16)


def _prep_edges(gidx, anode, n_nodes):
    """Host edge partitioning for one direction (identical geometry to v1)."""
    npc, nb, half, _, _ = _derive(n_nodes)
    n_all = ((n_nodes + 127) // 128) * 128
    dlo, dhi = half + 16, (n_all - half) + 16
    core = anode // npc
    local = anode - core * npc
    block = local // 128
    dstpos = local % 128
    hi = (gidx >= half).astype(np.int64)
    lidx = gidx - hi * half

    counts = np.zeros((NCORES, nb, 2), dtype=np.int64)
    np.add.at(counts, (core, block, hi), 1)
    nchunks = -(-counts // 128)  # ceil
    clo = int(nchunks[:, :, 0].max())
    chi = int(nchunks[:, :, 1].max())

    order = np.lexsort((lidx, hi, block, core))
    g_s, blk_s, hi_s, lidx_s, dp_s = (
        core[order], block[order], hi[order], lidx[order], dstpos[order])

    cmax = max(clo, chi)
    g1 = np.empty((NCORES, nb, 2, cmax * 128), dtype=np.int16)
    g1[:, :, 0, :] = dlo
    g1[:, :, 1, :] = dhi
    g2 = np.zeros((NCORES, nb, 2, cmax * 128), dtype=np.int16)
    dp = np.full((NCORES, nb, 2, cmax * 128), 200.0, dtype=ml_dtypes.bfloat16)

    flat_grp = (g_s * nb + blk_s) * 2 + hi_s
    idx_sorted = np.argsort(flat_grp, kind="stable")
    fg = flat_grp[idx_sorted]
    pos = np.arange(len(fg)) - np.concatenate(
        ([0], np.cumsum(np.bincount(fg, minlength=NCORES * nb * 2))))[fg]
    c_, b_, h_ = fg // (nb * 2), (fg // 2) % nb, fg % 2
    g1[c_, b_, h_, pos] = lidx_s[idx_sorted].astype(np.int16)
    g2[c_, b_, h_, pos] = (blk_s[idx_sorted] * 128 + dp_s[idx_sorted]).astype(np.int16)
    g2[:, :, :, :][g2 < 0] = 0
    padmask = np.ones((NCORES, nb, 2, cmax * 128), dtype=bool)
    padmask[c_, b_, h_, pos] = False
    bb = np.broadcast_to(np.arange(nb)[None, :, None, None] * 128,
                         (NCORES, nb, 2, cmax * 128))
    g2[padmask] = bb[padmask].astype(np.int16)
    dp[c_, b_, h_, pos] = dp_s[idx_sorted].astype(ml_dtypes.bfloat16)

    g1lo, g1hi = g1[:, :, 0, :clo * 128], g1[:, :, 1, :chi * 128]
    g2lo, g2hi = g2[:, :, 0, :clo * 128], g2[:, :, 1, :chi * 128]
    dplo, dphi = dp[:, :, 0, :clo * 128], dp[:, :, 1, :chi * 128]

    # gather idx streams per supergroup: [lo idxs | hi idxs] (block-major)
    g1_streams, g2_streams = [], []
    for c in range(NCORES):
        p1, p2 = [], []
        for bs in range(0, nb, KB):
            be = min(bs + KB, nb)
            p1 += [_pack_idx(g1lo[c, bs:be].ravel()), _pack_idx(g1hi[c, bs:be].ravel())]
            p2 += [_pack_idx(g2lo[c, bs:be].ravel()), _pack_idx(g2hi[c, bs:be].ravel())]
        g1_streams.append(np.concatenate(p1, axis=1))
        g2_streams.append(np.concatenate(p2, axis=1))
    # dstpos stream [128, nb*(clo+chi)] block-major, all lo chunks then all hi
    dpl = dplo.reshape(NCORES, nb, clo, 128).transpose(0, 3, 1, 2).reshape(NCORES, 128, nb * clo)
    dph = dphi.reshape(NCORES, nb, chi, 128).transpose(0, 3, 1, 2).reshape(NCORES, 128, nb * chi)
    dp_stream = np.concatenate([dpl, dph], axis=2)
    return clo, chi, np.stack(g1_streams), np.stack(g2_streams), np.ascontiguousarray(dp_stream)


def _build_program(n_nodes, clo_f, chi_f, clo_b, chi_b):
    npc, nb, half, trows_lo, trows_hi = _derive(n_nodes)
    npad = nb * 128
    n_all = ((n_nodes + 127) // 128) * 128
    npb = n_all // 128                     # projection node blocks
    dummy_lo = half + 16
    dummy_hi = (n_all - half) + 16
    hblocks = half // 128

    nc = bacc.Bacc(None, target_bir_lowering=False)
    inp = lambda name, shape, dt: nc.declare_dram_parameter(name, shape, dt, isOutput=False)
    xT = inp("xT", [512, n_all], bf16)
    xTo = inp("xTo", [512, npad], bf16)
    wall = inp("wall", [512, 528], bf16)
    wfuse = inp("wfuse", [512, 512], bf16)
    drow = inp("drow", [1, TSTRIDE], bf16)
    iota_in = inp("iota", [128, 128], bf16)
    ident = inp("ident", [128, 128], f32)
    identb = inp("identb", [128, 128], bf16)
    gb = inp("gb", [2, 512], f32)
    streams = {}
    for d, (clo, chi) in (("f", (clo_f, chi_f)), ("b", (clo_b, chi_b))):
        tc_d = nb * (clo + chi)
        streams["g1" + d] = inp("g1" + d, [128, tc_d * 8], i16)
        streams["g2" + d] = inp("g2" + d, [128, tc_d * 8], i16)
        streams["dp" + d] = inp("dp" + d, [128, tc_d], f32)
    out_d = nc.declare_dram_parameter("out", [npc, 512], f32, isOutput=True)

    tabs = {d: [nc.dram_tensor(f"tab{d}{h}", [tr, TSTRIDE], bf16)
                for h, tr in (("lo", trows_lo), ("hi", trows_hi))] for d in "fb"}
    adtab = nc.dram_tensor("adtab", [npad, 128], bf16)
    ccin = nc.dram_tensor("ccin", [2, 512], f32)
    ccout = nc.dram_tensor("ccout", [2, 512], f32, addr_space="Shared")

    with tile.TileContext(nc) as tc:
        with tc.tile_pool(name="const", bufs=1) as cpool:
            nc.gpsimd.load_library(library_config.mlp)
            wall_sb = cpool.tile([128, 4, 528], bf16)
            for k in range(4):
                nc.sync.dma_start(wall_sb[:, k, :], wall[k * 128:(k + 1) * 128, :])
            wf_sb = cpool.tile([128, 4, 512], bf16)
            for k in range(4):
                nc.sync.dma_start(wf_sb[:, k, :], wfuse[k * 128:(k + 1) * 128, :])
            iota_sb = cpool.tile([128, 128], bf16)
            nc.sync.dma_start(iota_sb[:], iota_in[:])
            idb_sb = cpool.tile([128, 128], bf16)
            nc.sync.dma_start(idb_sb[:], identb[:])
            gam_sb = cpool.tile([1, 512], f32)
            nc.sync.dma_start(gam_sb[:], gb[0:1, :])
            bet_sb = cpool.tile([1, 512], f32)
            nc.sync.dma_start(bet_sb[:], gb[1:2, :])
            ones_col = cpool.tile([128, 1], bf16)
            nc.vector.memset(ones_col[:], 1.0)
            ones1 = cpool.tile([1, 128], f32)
            nc.vector.memset(ones1[:], 1.0)
            # zero-fill table tail rows, then dummy rows
            zt = cpool.tile([128, TSTRIDE], bf16)
            nc.gpsimd.memset(zt[:], 0.0)
            for d in "fb":
                r = half
                while r < trows_lo:
                    n = min(128, trows_lo - r)
                    nc.sync.dma_start(tabs[d][0][r:r + n, :], zt[0:n, :])
                    r += n
                r = n_all - half
                while r < trows_hi:
                    n = min(128, trows_hi - r)
                    nc.sync.dma_start(tabs[d][1][r:r + n, :], zt[0:n, :])
                    r += n
                nc.sync.dma_start(tabs[d][0][dummy_lo:dummy_lo + 1, :], drow[:])
                nc.sync.dma_start(tabs[d][1][dummy_hi:dummy_hi + 1, :], drow[:])

            # persistent slabs
            obf = cpool.tile([128, nb, 256], bf16)
            fused = cpool.tile([128, nb, 512], bf16)

            # ---------------- projection (both dirs, one xT pass) ----------------
            with (tc.tile_pool(name="proj", bufs=2) as pj,
                  tc.tile_pool(name="pspj", bufs=2, space="PSUM") as pp):
                ngrp = (npb + 7) // 8
                for g in range(ngrp):
                    b0 = g * 8
                    nblk = min(8, npb - b0)
                    w = nblk * 128
                    xt = pj.tile([128, 4, XW], bf16, tag="xt")
                    nc.sync.dma_start(
                        xt[:, :, 0:w],
                        xT[:, b0 * 128:b0 * 128 + w]
                        .rearrange("(k p) n -> p k n", p=128))
                    stg = pj.tile([128, 8, 520], bf16, tag="stg")
                    for j in range(nblk):
                        psf = pp.tile([128, 260], f32, tag="psf", name=f"psf_{g}_{j}")
                        psb = pp.tile([128, 260], f32, tag="psb", name=f"psb_{g}_{j}")
                        for k in range(4):
                            nc.tensor.matmul(psf[:], xt[:, k, j * 128:(j + 1) * 128],
                                             wall_sb[:, k, 0:260],
                                             start=(k == 0), stop=(k == 3))
                        for k in range(4):
                            nc.tensor.matmul(psb[:], xt[:, k, j * 128:(j + 1) * 128],
                                             wall_sb[:, k, 260:520],
                                             start=(k == 0), stop=(k == 3))
                        nc.scalar.activation(stg[:, j, 0:260], psf[:], Act.Copy)
                        nc.vector.tensor_copy(stg[:, j, 260:520], psb[:])
                    # batched table writes (handle lo/hi straddle)
                    for d, c0 in (("f", 0), ("b", 260)):
                        j = 0
                        while j < nblk:
                            blk = b0 + j
                            if blk < hblocks:
                                nmax = min(nblk - j, hblocks - blk)
                                dst = tabs[d][0][blk * 128:(blk + nmax) * 128, 0:TROW]
                            else:
                                nmax = nblk - j
                                r0 = (blk - hblocks) * 128
                                dst = tabs[d][1][r0:r0 + nmax * 128, 0:TROW]
                            nc.sync.dma_start(
                                dst.rearrange("(j p) c -> p j c", p=128),
                                stg[:, j:j + nmax, c0:c0 + TROW])
                            j += nmax
                # local a_d table (own shard via xTo)
                ad_stage = pj.tile([128, nb, 8], bf16, tag="ad")
                ngo = (nb * 128 + XW - 1) // XW
                for g in range(ngo):
                    b0 = g * 8
                    nblk = min(8, nb - b0)
                    w = nblk * 128
                    xo = pj.tile([128, 4, XW], bf16, tag="xt")
                    nc.sync.dma_start(
                        xo[:, :, 0:w],
                        xTo[:, b0 * 128:b0 * 128 + w]
                        .rearrange("(k p) n -> p k n", p=128))
                    for j in range(nblk):
                        pa = pp.tile([128, 8], f32, tag="pa", bufs=1)
                        for k in range(4):
                            nc.tensor.matmul(pa[:], xo[:, k, j * 128:(j + 1) * 128],
                                             wall_sb[:, k, 520:528],
                                             start=(k == 0), stop=(k == 3))
                        nc.vector.tensor_copy(ad_stage[:, b0 + j, :], pa[:])
                nc.sync.dma_start(
                    adtab.rearrange("(b p) c -> p b c", p=128)[:, :, 0:8], ad_stage[:])

            # ---------------- edge passes + fusion ----------------
            # PSUM pools that live across both edge passes
            with (tc.tile_pool(name="psed", bufs=2, space="PSUM") as ppb,
                  tc.tile_pool(name="psfu", bufs=2, space="PSUM") as ppf,
                  tc.tile_pool(name="psst", bufs=1, space="PSUM") as pps,
                  tc.tile_pool(name="edges", bufs=1) as es):
                stat1 = pps.tile([1, 512], f32, name="stat1")
                stat2 = pps.tile([1, 512], f32, name="stat2")
                for d, clo, chi, adofs in (("f", clo_f, chi_f, 0),
                                           ("b", clo_b, chi_b, 4)):
                    tc_d = nb * (clo + chi)
                    dp_sb = es.tile([128, tc_d], f32, tag="dp" + d, name="dp" + d)
                    nc.sync.dma_start(dp_sb[:], streams["dp" + d][:])
                    ccm = max(clo_f, chi_f, clo_b, chi_b)
                    with (tc.tile_pool(name="est" + d, bufs=2) as est,
                          tc.tile_pool(name="ew" + d, bufs=2) as ew):
                        c1 = 0
                        for bs in range(0, nb, KB):
                            kbs = min(KB, nb - bs)
                            nlo, nhi = kbs * clo * 128, kbs * chi * 128
                            span = (nlo + nhi) // 16
                            g1s = est.tile([128, KB * (clo + chi) * 8], i16, tag="g1s")
                            nc.scalar.dma_start(g1s[:, 0:span],
                                                streams["g1" + d][:, c1:c1 + span])
                            g2s = est.tile([128, KB * (clo + chi) * 8], i16, tag="g2s")
                            nc.scalar.dma_start(g2s[:, 0:span],
                                                streams["g2" + d][:, c1:c1 + span])
                            c1 += span
                            gtl = est.tile([128, KB * clo, TSTRIDE], bf16, tag="gtl")
                            nc.gpsimd.dma_gather(
                                gtl[:, 0:kbs * clo, :], tabs[d][0][:],
                                g1s[:, 0:nlo // 16], num_idxs=nlo, num_idxs_reg=nlo,
                                elem_size=TSTRIDE, single_packet=False)
                            gth = est.tile([128, KB * chi, TSTRIDE], bf16, tag="gth")
                            nc.gpsimd.dma_gather(
                                gth[:, 0:kbs * chi, :], tabs[d][1][:],
                                g1s[:, nlo // 16:span], num_idxs=nhi, num_idxs_reg=nhi,
                                elem_size=TSTRIDE, single_packet=False)
                            at = est.tile([128, KB * (clo + chi), 128], bf16, tag="at")
                            nc.gpsimd.dma_gather(
                                at[:, 0:kbs * (clo + chi), :], adtab[:],
                                g2s[:, 0:span], num_idxs=nlo + nhi,
                                num_idxs_reg=nlo + nhi, elem_size=128,
                                single_packet=False)
                            for j in range(kbs):
                                b = bs + j
                                pb = ppb.tile([128, 260], f32, tag="pb",
                                              name=f"pb{d}_{b}")
                                first = True
                                for hname, cc, gt, atof, dpof in (
                                        ("lo", clo, gtl, j * clo, b * clo),
                                        ("hi", chi, gth,
                                         kbs * clo + j * chi, nb * clo + b * chi)):
                                    if cc == 0:
                                        continue
                                    sl = slice(j * cc, (j + 1) * cc)
                                    et = ew.tile([128, ccm, 4], bf16, tag="et")
                                    nc.vector.tensor_tensor(
                                        et[:, 0:cc, :], gt[:, sl, 256:260],
                                        at[:, atof:atof + cc, adofs:adofs + 4], Alu.add)
                                    lt = ew.tile([128, ccm * 4], f32, tag="lt")
                                    nc.vector.scalar_tensor_tensor(
                                        lt[:, 0:cc * 4],
                                        et[:, 0:cc, :].rearrange("p c h -> p (c h)"),
                                        NEG_SLOPE,
                                        et[:, 0:cc, :].rearrange("p c h -> p (c h)"),
                                        Alu.mult, Alu.max)
                                    mt = ew.tile([128, ccm, 260], bf16, tag="mt")
                                    nc.scalar.activation(
                                        mt[:, 0:cc, 256:260],
                                        lt[:, 0:cc * 4].rearrange(
                                            "p (c h) -> p c h", h=4), Act.Exp)
                                    nc.vector.tensor_tensor(
                                        mt[:, 0:cc, 0:256].rearrange(
                                            "p c (f h) -> p c f h", h=4),
                                        gt[:, sl, 0:256].rearrange(
                                            "p c (f h) -> p c f h", h=4),
                                        mt[:, 0:cc, 256:260].unsqueeze(2)
                                        .broadcast_to((128, cc, 64, 4)),
                                        Alu.mult)
                                    st = ew.tile([128, ccm, 128], bf16, tag="st")
                                    for k in range(cc):
                                        nc.vector.tensor_scalar(
                                            st[:, k, :], iota_sb[:],
                                            dp_sb[:, dpof + k:dpof + k + 1], None,
                                            op0=Alu.is_equal)
                                    for k in range(cc):
                                        last = (hname == "hi" or chi == 0) and k == cc - 1
                                        nc.tensor.matmul(pb[:], st[:, k, :], mt[:, k, :],
                                                         start=first, stop=last)
                                        first = False
                                dn = ew.tile([128, 4], f32, tag="dn")
                                nc.vector.tensor_scalar_add(dn[:], pb[:, 256:260], 1e-16)
                                rc = ew.tile([128, 4], f32, tag="rc")
                                nc.vector.reciprocal(rc[:], dn[:])
                                if d == "f":
                                    nc.vector.tensor_tensor(
                                        obf[:, b, :].rearrange("p (f h) -> p f h", h=4),
                                        pb[:, 0:256].rearrange("p (f h) -> p f h", h=4),
                                        rc[:].unsqueeze(1)
                                        .broadcast_to((128, 64, 4)),
                                        Alu.mult)
                                else:
                                    obb = ew.tile([128, 256], bf16, tag="obb")
                                    nc.vector.tensor_tensor(
                                        obb[:].rearrange("p (f h) -> p f h", h=4),
                                        pb[:, 0:256].rearrange("p (f h) -> p f h", h=4),
                                        rc[:].unsqueeze(1)
                                        .broadcast_to((128, 64, 4)),
                                        Alu.mult)
                                    # -------- per-block fusion --------
                                    ct = ew.tile([128, 4, 128], bf16, tag="ct")
                                    for k, src in enumerate(
                                            (obf[:, b, 0:128], obf[:, b, 128:256],
                                             obb[:, 0:128], obb[:, 128:256])):
                                        pt = ppb.tile([128, 128], bf16, tag="pt", bufs=1)
                                        nc.tensor.transpose(pt[:], src, idb_sb[:])
                                        nc.scalar.activation(ct[:, k, :], pt[:], Act.Copy)
                                    pf = ppf.tile([128, 512], f32, tag="pf",
                                                  name=f"pf_{b}", bufs=1)
                                    for k in range(4):
                                        nc.tensor.matmul(pf[:], ct[:, k, :],
                                                         wf_sb[:, k, :],
                                                         start=(k == 0), stop=(k == 3))
                                    nc.scalar.activation(fused[:, b, :], pf[:], Act.Copy)
                                    sq = ew.tile([128, 512], bf16, tag="sq")
                                    nc.vector.tensor_tensor(sq[:], fused[:, b, :],
                                                            fused[:, b, :], Alu.mult)
                                    nc.tensor.matmul(stat1[:], ones_col[:],
                                                     fused[:, b, :],
                                                     start=(b == 0), stop=(b == nb - 1))
                                    nc.tensor.matmul(stat2[:], ones_col[:], sq[:],
                                                     start=(b == 0), stop=(b == nb - 1))

                # ---------------- BN tail ----------------
                with tc.tile_pool(name="tail", bufs=1) as tl:
                    stat_sa = tl.tile([1, 512], f32)
                    nc.vector.tensor_copy(stat_sa[:], stat1[:])
                    stat_sbb = tl.tile([1, 512], f32)
                    nc.vector.tensor_copy(stat_sbb[:], stat2[:])
                    nc.sync.dma_start(ccin[0:1, :], stat_sa[:])
                    nc.sync.dma_start(ccin[1:2, :], stat_sbb[:])
                    sga = tl.tile([1, 512], f32)
                    sgb = tl.tile([1, 512], f32)
                    if USE_CC:
                        nc.gpsimd.collective_compute(
                            "AllReduce", Alu.add,
                            replica_groups=[list(range(NCORES))],
                            ins=[ccin[:]], outs=[ccout[:]])
                        nc.sync.dma_start(sga[:], ccout[0:1, :])
                        nc.sync.dma_start(sgb[:], ccout[1:2, :])
                    else:
                        nc.sync.dma_start(sga[:], ccin[0:1, :])
                        nc.sync.dma_start(sgb[:], ccin[1:2, :])
                    m = tl.tile([1, 512], f32)
                    nc.vector.tensor_scalar_mul(m[:], sga[:], 1.0 / n_nodes)
                    e2 = tl.tile([1, 512], f32)
                    nc.vector.tensor_scalar_mul(e2[:], sgb[:], 1.0 / n_nodes)
                    var = tl.tile([1, 512], f32)
                    nc.vector.scalar_tensor_tensor(
                        var[:], m[:], 1.0, m[:], Alu.mult, Alu.mult)
                    nc.vector.tensor_tensor(var[:], e2[:], var[:], Alu.subtract)
                    nc.vector.tensor_scalar_add(var[:], var[:], BN_EPS)
                    sd = tl.tile([1, 512], f32)
                    nc.scalar.activation(sd[:], var[:], Act.Sqrt)
                    rs = tl.tile([1, 512], f32)
                    nc.vector.reciprocal(rs[:], sd[:])
                    A = tl.tile([1, 512], f32)
                    nc.vector.tensor_tensor(A[:], rs[:], gam_sb[:], Alu.mult)
                    mA = tl.tile([1, 512], f32)
                    nc.vector.tensor_tensor(mA[:], m[:], A[:], Alu.mult)
                    B = tl.tile([1, 512], f32)
                    nc.vector.tensor_tensor(B[:], bet_sb[:], mA[:], Alu.subtract)
                    pA = ppf.tile([128, 512], f32, tag="pA", bufs=1)
                    nc.tensor.matmul(pA[:], ones1[:], A[:])
                    pB = ppf.tile([128, 512], f32, tag="pB", bufs=1)
                    nc.tensor.matmul(pB[:], ones1[:], B[:])
                    pA_sb = tl.tile([128, 512], f32)
                    nc.scalar.activation(pA_sb[:], pA[:], Act.Copy)
                    pB_sb = tl.tile([128, 512], f32)
                    nc.scalar.activation(pB_sb[:], pB[:], Act.Copy)
                    with tc.tile_pool(name="norm", bufs=3) as nw:
                        for b in range(nb):
                            t1 = nw.tile([128, 512], f32, tag="t1")
                            eng = nc.vector if b % 2 == 0 else nc.gpsimd
                            eng.tensor_tensor(t1[:], fused[:, b, :], pA_sb[:], Alu.mult)
                            eng.tensor_tensor(t1[:], t1[:], pB_sb[:], Alu.add)
                            eng.tensor_scalar_max(t1[:], t1[:], 0.0)
                            rows = min(128, npc - b * 128)
                            nc.sync.dma_start(out_d[b * 128:b * 128 + rows, :],
                                              t1[0:rows, :])
    nc.compile()
    return nc


def kernel(**inputs):
    x = np.asarray(inputs["x"], dtype=np.float32)
    ei = np.asarray(inputs["edge_index"])
    n_nodes, D = x.shape
    npc, nb, half, trows_lo, trows_hi = _derive(n_nodes)
    n_all = ((n_nodes + 127) // 128) * 128
    npad = nb * 128

    def g(name):
        return np.asarray(inputs[name], dtype=np.float32)

    W_f, W_b = g("W_f"), g("W_b")
    asf, adf = g("att_src_f"), g("att_dst_f")
    asb, adb = g("att_src_b"), g("att_dst_b")
    W_fuse = g("W_fuse")
    gamma, beta = g("bn_gamma"), g("bn_beta")

    # (c,h)-interleaved weight layout: col c*4+h <- W[:, h, c]
    wall = np.zeros((512, 528), dtype=np.float32)
    wall[:, 0:256] = W_f.transpose(0, 2, 1).reshape(512, 256)
    wall[:, 256:260] = np.einsum("dhc,hc->dh", W_f, asf)
    wall[:, 260:516] = W_b.transpose(0, 2, 1).reshape(512, 256)
    wall[:, 516:520] = np.einsum("dhc,hc->dh", W_b, asb)
    wall[:, 520:524] = np.einsum("dhc,hc->dh", W_f, adf)
    wall[:, 524:528] = np.einsum("dhc,hc->dh", W_b, adb)

    # W_fuse rows permuted to the (c,h)-interleaved combined layout
    wfp = np.zeros_like(W_fuse)
    hc = np.arange(256)
    h_, c_ = hc // 64, hc % 64
    wfp[c_ * 4 + h_, :] = W_fuse[hc, :]
    wfp[256 + c_ * 4 + h_, :] = W_fuse[256 + hc, :]

    xT = np.zeros((512, n_all), dtype=ml_dtypes.bfloat16)
    xT[:, :n_nodes] = x.T
    drow = np.zeros((1, TSTRIDE), dtype=ml_dtypes.bfloat16)
    drow[0, 256:260] = DUMMY_AS
    iota = np.broadcast_to(np.arange(128), (128, 128)).astype(ml_dtypes.bfloat16)
    ident = np.eye(128, dtype=np.float32)
    gb = np.stack([gamma, beta]).astype(np.float32)

    src, dst = ei[0].astype(np.int64), ei[1].astype(np.int64)
    clo_f, chi_f, g1f, g2f, dpf = _prep_edges(src, dst, n_nodes)
    clo_b, chi_b, g1b, g2b, dpb = _prep_edges(dst, src, n_nodes)

    nc = _build_program(n_nodes, clo_f, chi_f, clo_b, chi_b)

    in_maps = []
    for c in range(NCORES):
        xTo = np.zeros((512, npad), dtype=ml_dtypes.bfloat16)
        xTo[:, :npc] = x.T[:, c * npc:(c + 1) * npc]
        in_maps.append({
            "xT": xT, "xTo": xTo,
            "wall": wall.astype(ml_dtypes.bfloat16),
            "wfuse": wfp.astype(ml_dtypes.bfloat16),
            "drow": drow, "iota": iota, "ident": ident,
            "identb": ident.astype(ml_dtypes.bfloat16), "gb": gb,
            "g1f": g1f[c], "g2f": g2f[c], "dpf": dpf[c].astype(np.float32),
            "g1b": g1b[c], "g2b": g2b[c], "dpb": dpb[c].astype(np.float32),
        })
    kernel.last_nc = nc
    res = run_bass_kernel_spmd(nc, in_maps, list(range(NCORES)))
    out = np.concatenate([np.asarray(res.results[c]["out"]) for c in range(NCORES)], axis=0)
    return out[:n_nodes].astype(np.float32)


if __name__ == "__main__":
    pass


# revision 13
# speedup vs baseline: 2.0358x; 1.0887x over previous
"""Bidirectional GATConv + fusion + BatchNorm + ReLU on 8 Trainium2 cores.

v2 design (cost-model driven):
  - DMA descriptor-time is the hard floor (all transfers serialize on the
    DMA_ENGINES device). Tables shrunk to 520B rows (260 bf16 cols,
    (c,h)-interleaved), a_d table to 32B rows, xT loaded with 2KB
    descriptors, no `combined` DRAM round-trip.
  - Edge inner loop: one batched DVE tensor_tensor for the alpha-weighting
    (ext broadcast rides a non-last stride-0 dim so 2x bf16 packing holds),
    one tensor_scalar one-hot per chunk, exp writes straight into the
    message tile's denominator columns so one 260-row matmul per chunk
    scatters messages + denominators.
  - Fusion per dst-block via PE transposes (no DRAM transpose), BN stats
    via ones-matmul PSUM accumulation, 4KB AllReduce, normalize+ReLU
    split across DVE/Pool.
Biases provably cancel through BatchNorm and are dropped.
"""
import sys

sys.path.insert(0, "/opt/trn_rl_repo")

import numpy as np
import ml_dtypes

import concourse.bass as bass
import concourse.bacc as bacc
import concourse.mybir as mybir
from concourse import tile
from concourse import library_config
from concourse.bass_utils import run_bass_kernel_spmd

bf16 = mybir.dt.bfloat16
f32 = mybir.dt.float32
i16 = mybir.dt.int16
Alu = mybir.AluOpType
Act = mybir.ActivationFunctionType

NCORES = 8
USE_CC = __import__("os").environ.get("NO_CC", "0") != "1"
KB = 2          # dst blocks per gather supergroup
NEG_SLOPE = 0.2
BN_EPS = 1e-5
DUMMY_AS = -60.0
TROW = 260      # written cols: 256 (c,h)-interleaved h + 4 a_s
TSTRIDE = 384   # physical table row stride (768B, gather elem must be 256B-mult)
XW = 1024       # xT load column batch (2KB descriptors)


def _derive(n_nodes):
    npc = n_nodes // NCORES
    nb = (npc + 127) // 128
    half = ((n_nodes // 2) // 128) * 128
    trows_lo = half + 64            # dummy row at half+16
    trows_hi = (n_nodes - half) + NCORES * 16 + 64
    return npc, nb, half, trows_lo, trows_hi


def _pack_idx(arr):
    """int16 [n] (n%16==0) -> [128, n/16] wrapped in 16 partitions, replicated per Q7 core."""
    a = arr.reshape(-1, 16).T
    return np.tile(a, (8, 1)).astype(np.int16)


def _prep_edges(gidx, anode, n_nodes):
    """Host edge partitioning for one direction (identical geometry to v1)."""
    npc, nb, half, _, _ = _derive(n_nodes)
    n_all = ((n_nodes + 127) // 128) * 128
    dlo, dhi = half + 16, (n_all - half) + 16
    core = anode // npc
    local = anode - core * npc
    block = local // 128
    dstpos = local % 128
    hi = (gidx >= half).astype(np.int64)
    lidx = gidx - hi * half

    counts = np.zeros((NCORES, nb, 2), dtype=np.int64)
    np.add.at(counts, (core, block, hi), 1)
    nchunks = -(-counts // 128)  # ceil
    clo = int(nchunks[:, :, 0].max())
    chi = int(nchunks[:, :, 1].max())

    order = np.lexsort((lidx, hi, block, core))
    g_s, blk_s, hi_s, lidx_s, dp_s = (
        core[order], block[order], hi[order], lidx[order], dstpos[order])

    cmax = max(clo, chi)
    g1 = np.empty((NCORES, nb, 2, cmax * 128), dtype=np.int16)
    g1[:, :, 0, :] = dlo
    g1[:, :, 1, :] = dhi
    g2 = np.zeros((NCORES, nb, 2, cmax * 128), dtype=np.int16)
    dp = np.full((NCORES, nb, 2, cmax * 128), 200.0, dtype=ml_dtypes.bfloat16)

    flat_grp = (g_s * nb + blk_s) * 2 + hi_s
    idx_sorted = np.argsort(flat_grp, kind="stable")
    fg = flat_grp[idx_sorted]
    pos = np.arange(len(fg)) - np.concatenate(
        ([0], np.cumsum(np.bincount(fg, minlength=NCORES * nb * 2))))[fg]
    c_, b_, h_ = fg // (nb * 2), (fg // 2) % nb, fg % 2
    g1[c_, b_, h_, pos] = lidx_s[idx_sorted].astype(np.int16)
    g2[c_, b_, h_, pos] = (blk_s[idx_sorted] * 128 + dp_s[idx_sorted]).astype(np.int16)
    g2[:, :, :, :][g2 < 0] = 0
    padmask = np.ones((NCORES, nb, 2, cmax * 128), dtype=bool)
    padmask[c_, b_, h_, pos] = False
    bb = np.broadcast_to(np.arange(nb)[None, :, None, None] * 128,
                         (NCORES, nb, 2, cmax * 128))
    g2[padmask] = bb[padmask].astype(np.int16)
    dp[c_, b_, h_, pos] = dp_s[idx_sorted].astype(ml_dtypes.bfloat16)

    g1lo, g1hi = g1[:, :, 0, :clo * 128], g1[:, :, 1, :chi * 128]
    g2lo, g2hi = g2[:, :, 0, :clo * 128], g2[:, :, 1, :chi * 128]
    dplo, dphi = dp[:, :, 0, :clo * 128], dp[:, :, 1, :chi * 128]

    # gather idx streams per supergroup: [lo idxs | hi idxs] (block-major)
    g1_streams, g2_streams = [], []
    for c in range(NCORES):
        p1, p2 = [], []
        for bs in range(0, nb, KB):
            be = min(bs + KB, nb)
            p1 += [_pack_idx(g1lo[c, bs:be].ravel()), _pack_idx(g1hi[c, bs:be].ravel())]
            p2 += [_pack_idx(g2lo[c, bs:be].ravel()), _pack_idx(g2hi[c, bs:be].ravel())]
        g1_streams.append(np.concatenate(p1, axis=1))
        g2_streams.append(np.concatenate(p2, axis=1))
    # dstpos stream [128, nb*(clo+chi)] block-major, all lo chunks then all hi
    dpl = dplo.reshape(NCORES, nb, clo, 128).transpose(0, 3, 1, 2).reshape(NCORES, 128, nb * clo)
    dph = dphi.reshape(NCORES, nb, chi, 128).transpose(0, 3, 1, 2).reshape(NCORES, 128, nb * chi)
    dp_stream = np.concatenate([dpl, dph], axis=2)
    return clo, chi, np.stack(g1_streams), np.stack(g2_streams), np.ascontiguousarray(dp_stream)




def _balance(src, dst, n_nodes):
    """Assign nodes to (core, block) bins so every bin's 4 incident-edge
    counts (f_lo, f_hi, b_lo, b_hi) are <= 1024 (8 chunks of 128), keeping
    each node's half-class so neighbor half membership stays fixed.
    Returns newpos[old_id] -> new_id."""
    import time as _time
    npc, nb, half, _, _ = _derive(n_nodes)
    nbins = NCORES * nb
    cap = np.full(nbins, 128, dtype=np.int64)
    for c in range(NCORES):
        cap[c * nb + nb - 1] = npc - (nb - 1) * 128
    is_lo_src = src < half
    is_lo_dst = dst < half
    d = np.zeros((n_nodes, 4), dtype=np.int64)
    np.add.at(d[:, 0], dst[is_lo_src], 1)
    np.add.at(d[:, 1], dst[~is_lo_src], 1)
    np.add.at(d[:, 2], src[is_lo_dst], 1)
    np.add.at(d[:, 3], src[~is_lo_dst], 1)
    cap_lo = np.zeros(nbins, dtype=np.int64)
    for c in range(NCORES):
        for b in range(nb):
            i = c * nb + b
            start = c * npc + b * 128
            cap_lo[i] = max(0, min(start + cap[i], half) - start)
    cap_hi = cap - cap_lo

    rng = np.random.default_rng(0)
    sums = np.zeros((nbins, 4), dtype=np.int64)
    slots_lo, slots_hi = cap_lo.copy(), cap_hi.copy()
    assign = np.empty(n_nodes, dtype=np.int64)
    order_nodes = np.argsort(-d.sum(1), kind="stable")
    for n in order_nodes:
        slots = slots_lo if n < half else slots_hi
        feas = slots > 0
        cand = sums[feas] + d[n]
        score = cand.max(1) / 1024.0 - 1e-6 * slots[feas]
        j = np.flatnonzero(feas)[np.argmin(score)]
        assign[n] = j
        sums[j] += d[n]
        slots[j] -= 1
    # swap-repair toward all bins <= 1024 on all 4 dims
    half_class = np.arange(n_nodes) < half
    members = [np.flatnonzero(assign == i).tolist() for i in range(nbins)]
    t0 = _time.time()
    for it in range(200000):
        over = np.argwhere(sums > 1024)
        if len(over) == 0 or _time.time() - t0 > 90:
            break
        A, dim = over[rng.integers(len(over))]
        n1 = members[A][rng.integers(len(members[A]))]
        h1 = half_class[n1]
        for _ in range(60):
            B = int(rng.integers(nbins))
            if B == A or not members[B]:
                continue
            k2 = int(rng.integers(len(members[B])))
            n2 = members[B][k2]
            if half_class[n2] != h1:
                continue
            dA = sums[A] - d[n1] + d[n2]
            dB = sums[B] - d[n2] + d[n1]
            if (dA <= np.maximum(sums[A], 1024)).all() and dA[dim] < sums[A][dim] \
               and (dB <= 1024).all():
                sums[A] = dA
                sums[B] = dB
                members[A].remove(n1)
                members[B][k2] = n1
                members[A].append(n2)
                assign[n1], assign[n2] = B, A
                break
    # slots within bins: lo-class nodes take the bin's lo prefix
    newpos = np.empty(n_nodes, dtype=np.int64)
    for i in range(nbins):
        c, b = divmod(i, nb)
        start = c * npc + b * 128
        mem = np.array(members[i], dtype=np.int64)
        lo_m = mem[half_class[mem]]
        hi_m = mem[~half_class[mem]]
        newpos[lo_m] = start + np.arange(len(lo_m))
        newpos[hi_m] = start + cap_lo[i] + np.arange(len(hi_m))
    return newpos

def _build_program(n_nodes, clo_f, chi_f, clo_b, chi_b):
    npc, nb, half, trows_lo, trows_hi = _derive(n_nodes)
    npad = nb * 128
    n_all = ((n_nodes + 127) // 128) * 128
    npb = n_all // 128                     # projection node blocks
    dummy_lo = half + 16
    dummy_hi = (n_all - half) + 16
    hblocks = half // 128

    nc = bacc.Bacc(None, target_bir_lowering=False)
    inp = lambda name, shape, dt: nc.declare_dram_parameter(name, shape, dt, isOutput=False)
    xT = inp("xT", [512, n_all], bf16)
    xTo = inp("xTo", [512, npad], bf16)
    wall = inp("wall", [512, 528], bf16)
    wfuse = inp("wfuse", [512, 512], bf16)
    drow = inp("drow", [1, TSTRIDE], bf16)
    iota_in = inp("iota", [128, 128], bf16)
    ident = inp("ident", [128, 128], f32)
    identb = inp("identb", [128, 128], bf16)
    gb = inp("gb", [2, 512], f32)
    streams = {}
    for d, (clo, chi) in (("f", (clo_f, chi_f)), ("b", (clo_b, chi_b))):
        tc_d = nb * (clo + chi)
        streams["g1" + d] = inp("g1" + d, [128, tc_d * 8], i16)
        streams["g2" + d] = inp("g2" + d, [128, tc_d * 8], i16)
        streams["dp" + d] = inp("dp" + d, [128, tc_d], f32)
    out_d = nc.declare_dram_parameter("out", [npc, 512], f32, isOutput=True)

    tabs = {d: [nc.dram_tensor(f"tab{d}{h}", [tr, TSTRIDE], bf16)
                for h, tr in (("lo", trows_lo), ("hi", trows_hi))] for d in "fb"}
    adtab = nc.dram_tensor("adtab", [npad, 128], bf16)
    ccin = nc.dram_tensor("ccin", [2, 512], f32)
    ccout = nc.dram_tensor("ccout", [2, 512], f32, addr_space="Shared")

    with tile.TileContext(nc) as tc:
        with tc.tile_pool(name="const", bufs=1) as cpool:
            nc.gpsimd.load_library(library_config.mlp)
            wall_sb = cpool.tile([128, 4, 528], bf16)
            for k in range(4):
                nc.sync.dma_start(wall_sb[:, k, :], wall[k * 128:(k + 1) * 128, :])
            wf_sb = cpool.tile([128, 4, 512], bf16)
            for k in range(4):
                nc.sync.dma_start(wf_sb[:, k, :], wfuse[k * 128:(k + 1) * 128, :])
            iota_sb = cpool.tile([128, 128], bf16)
            nc.sync.dma_start(iota_sb[:], iota_in[:])
            idb_sb = cpool.tile([128, 128], bf16)
            nc.sync.dma_start(idb_sb[:], identb[:])
            gam_sb = cpool.tile([1, 512], f32)
            nc.sync.dma_start(gam_sb[:], gb[0:1, :])
            bet_sb = cpool.tile([1, 512], f32)
            nc.sync.dma_start(bet_sb[:], gb[1:2, :])
            ones_col = cpool.tile([128, 1], bf16)
            nc.vector.memset(ones_col[:], 1.0)
            ones1 = cpool.tile([1, 128], f32)
            nc.vector.memset(ones1[:], 1.0)
            # zero-fill table tail rows, then dummy rows
            zt = cpool.tile([128, TSTRIDE], bf16)
            nc.gpsimd.memset(zt[:], 0.0)
            for d in "fb":
                r = half
                while r < trows_lo:
                    n = min(128, trows_lo - r)
                    nc.sync.dma_start(tabs[d][0][r:r + n, :], zt[0:n, :])
                    r += n
                r = n_all - half
                while r < trows_hi:
                    n = min(128, trows_hi - r)
                    nc.sync.dma_start(tabs[d][1][r:r + n, :], zt[0:n, :])
                    r += n
                nc.sync.dma_start(tabs[d][0][dummy_lo:dummy_lo + 1, :], drow[:])
                nc.sync.dma_start(tabs[d][1][dummy_hi:dummy_hi + 1, :], drow[:])

            # persistent slabs
            obf = cpool.tile([128, nb, 256], bf16)
            fused = cpool.tile([128, nb, 512], bf16)

            # ---------------- projection (both dirs, one xT pass) ----------------
            with (tc.tile_pool(name="proj", bufs=2) as pj,
                  tc.tile_pool(name="pspj", bufs=2, space="PSUM") as pp):
                ngrp = (npb + 7) // 8
                for g in range(ngrp):
                    b0 = g * 8
                    nblk = min(8, npb - b0)
                    w = nblk * 128
                    xt = pj.tile([128, 4, XW], bf16, tag="xt")
                    nc.sync.dma_start(
                        xt[:, :, 0:w],
                        xT[:, b0 * 128:b0 * 128 + w]
                        .rearrange("(k p) n -> p k n", p=128))
                    stg = pj.tile([128, 8, 520], bf16, tag="stg")
                    for j in range(nblk):
                        psf = pp.tile([128, 260], f32, tag="psf", name=f"psf_{g}_{j}")
                        psb = pp.tile([128, 260], f32, tag="psb", name=f"psb_{g}_{j}")
                        for k in range(4):
                            nc.tensor.matmul(psf[:], xt[:, k, j * 128:(j + 1) * 128],
                                             wall_sb[:, k, 0:260],
                                             start=(k == 0), stop=(k == 3))
                        for k in range(4):
                            nc.tensor.matmul(psb[:], xt[:, k, j * 128:(j + 1) * 128],
                                             wall_sb[:, k, 260:520],
                                             start=(k == 0), stop=(k == 3))
                        nc.scalar.activation(stg[:, j, 0:260], psf[:], Act.Copy)
                        nc.vector.tensor_copy(stg[:, j, 260:520], psb[:])
                    # batched table writes (handle lo/hi straddle)
                    for d, c0 in (("f", 0), ("b", 260)):
                        j = 0
                        while j < nblk:
                            blk = b0 + j
                            if blk < hblocks:
                                nmax = min(nblk - j, hblocks - blk)
                                dst = tabs[d][0][blk * 128:(blk + nmax) * 128, 0:TROW]
                            else:
                                nmax = nblk - j
                                r0 = (blk - hblocks) * 128
                                dst = tabs[d][1][r0:r0 + nmax * 128, 0:TROW]
                            nc.sync.dma_start(
                                dst.rearrange("(j p) c -> p j c", p=128),
                                stg[:, j:j + nmax, c0:c0 + TROW])
                            j += nmax
                # local a_d table (own shard via xTo)
                ad_stage = pj.tile([128, nb, 8], bf16, tag="ad")
                ngo = (nb * 128 + XW - 1) // XW
                for g in range(ngo):
                    b0 = g * 8
                    nblk = min(8, nb - b0)
                    w = nblk * 128
                    xo = pj.tile([128, 4, XW], bf16, tag="xt")
                    nc.sync.dma_start(
                        xo[:, :, 0:w],
                        xTo[:, b0 * 128:b0 * 128 + w]
                        .rearrange("(k p) n -> p k n", p=128))
                    for j in range(nblk):
                        pa = pp.tile([128, 8], f32, tag="pa", bufs=1)
                        for k in range(4):
                            nc.tensor.matmul(pa[:], xo[:, k, j * 128:(j + 1) * 128],
                                             wall_sb[:, k, 520:528],
                                             start=(k == 0), stop=(k == 3))
                        nc.vector.tensor_copy(ad_stage[:, b0 + j, :], pa[:])
                nc.sync.dma_start(
                    adtab.rearrange("(b p) c -> p b c", p=128)[:, :, 0:8], ad_stage[:])

            # ---------------- edge passes + fusion ----------------
            # PSUM pools that live across both edge passes
            with (tc.tile_pool(name="psed", bufs=2, space="PSUM") as ppb,
                  tc.tile_pool(name="psfu", bufs=2, space="PSUM") as ppf,
                  tc.tile_pool(name="psst", bufs=1, space="PSUM") as pps,
                  tc.tile_pool(name="edges", bufs=1) as es):
                stat1 = pps.tile([1, 512], f32, name="stat1")
                stat2 = pps.tile([1, 512], f32, name="stat2")
                for d, clo, chi, adofs in (("f", clo_f, chi_f, 0),
                                           ("b", clo_b, chi_b, 4)):
                    tc_d = nb * (clo + chi)
                    dp_sb = es.tile([128, tc_d], f32, tag="dp" + d, name="dp" + d)
                    nc.sync.dma_start(dp_sb[:], streams["dp" + d][:])
                    ccm = max(clo_f, chi_f, clo_b, chi_b)
                    with (tc.tile_pool(name="est" + d, bufs=2) as est,
                          tc.tile_pool(name="ew" + d, bufs=2) as ew):
                        c1 = 0
                        for bs in range(0, nb, KB):
                            kbs = min(KB, nb - bs)
                            nlo, nhi = kbs * clo * 128, kbs * chi * 128
                            span = (nlo + nhi) // 16
                            g1s = est.tile([128, KB * (clo + chi) * 8], i16, tag="g1s")
                            nc.scalar.dma_start(g1s[:, 0:span],
                                                streams["g1" + d][:, c1:c1 + span])
                            g2s = est.tile([128, KB * (clo + chi) * 8], i16, tag="g2s")
                            nc.scalar.dma_start(g2s[:, 0:span],
                                                streams["g2" + d][:, c1:c1 + span])
                            c1 += span
                            gtl = est.tile([128, KB * clo, TSTRIDE], bf16, tag="gtl")
                            nc.gpsimd.dma_gather(
                                gtl[:, 0:kbs * clo, :], tabs[d][0][:],
                                g1s[:, 0:nlo // 16], num_idxs=nlo, num_idxs_reg=nlo,
                                elem_size=TSTRIDE, single_packet=False)
                            gth = est.tile([128, KB * chi, TSTRIDE], bf16, tag="gth")
                            nc.gpsimd.dma_gather(
                                gth[:, 0:kbs * chi, :], tabs[d][1][:],
                                g1s[:, nlo // 16:span], num_idxs=nhi, num_idxs_reg=nhi,
                                elem_size=TSTRIDE, single_packet=False)
                            at = est.tile([128, KB * (clo + chi), 128], bf16, tag="at")
                            nc.gpsimd.dma_gather(
                                at[:, 0:kbs * (clo + chi), :], adtab[:],
                                g2s[:, 0:span], num_idxs=nlo + nhi,
                                num_idxs_reg=nlo + nhi, elem_size=128,
                                single_packet=False)
                            for j in range(kbs):
                                b = bs + j
                                pb = ppb.tile([128, 260], f32, tag="pb",
                                              name=f"pb{d}_{b}")
                                first = True
                                for hname, cc, gt, atof, dpof in (
                                        ("lo", clo, gtl, j * clo, b * clo),
                                        ("hi", chi, gth,
                                         kbs * clo + j * chi, nb * clo + b * chi)):
                                    if cc == 0:
                                        continue
                                    sl = slice(j * cc, (j + 1) * cc)
                                    et = ew.tile([128, ccm, 4], bf16, tag="et")
                                    nc.vector.tensor_tensor(
                                        et[:, 0:cc, :], gt[:, sl, 256:260],
                                        at[:, atof:atof + cc, adofs:adofs + 4], Alu.add)
                                    lt = ew.tile([128, ccm * 4], f32, tag="lt")
                                    nc.vector.scalar_tensor_tensor(
                                        lt[:, 0:cc * 4],
                                        et[:, 0:cc, :].rearrange("p c h -> p (c h)"),
                                        NEG_SLOPE,
                                        et[:, 0:cc, :].rearrange("p c h -> p (c h)"),
                                        Alu.mult, Alu.max)
                                    mt = ew.tile([128, ccm, 260], bf16, tag="mt")
                                    nc.scalar.activation(
                                        mt[:, 0:cc, 256:260],
                                        lt[:, 0:cc * 4].rearrange(
                                            "p (c h) -> p c h", h=4), Act.Exp)
                                    nc.vector.tensor_tensor(
                                        mt[:, 0:cc, 0:256].rearrange(
                                            "p c (f h) -> p c f h", h=4),
                                        gt[:, sl, 0:256].rearrange(
                                            "p c (f h) -> p c f h", h=4),
                                        mt[:, 0:cc, 256:260].unsqueeze(2)
                                        .broadcast_to((128, cc, 64, 4)),
                                        Alu.mult)
                                    st = ew.tile([128, ccm, 128], bf16, tag="st")
                                    for k in range(cc):
                                        nc.vector.tensor_scalar(
                                            st[:, k, :], iota_sb[:],
                                            dp_sb[:, dpof + k:dpof + k + 1], None,
                                            op0=Alu.is_equal)
                                    for k in range(cc):
                                        last = (hname == "hi" or chi == 0) and k == cc - 1
                                        nc.tensor.matmul(pb[:], st[:, k, :], mt[:, k, :],
                                                         start=first, stop=last)
                                        first = False
                                dn = ew.tile([128, 4], f32, tag="dn")
                                nc.vector.tensor_scalar_add(dn[:], pb[:, 256:260], 1e-16)
                                rc = ew.tile([128, 4], f32, tag="rc")
                                nc.vector.reciprocal(rc[:], dn[:])
                                if d == "f":
                                    nc.vector.tensor_tensor(
                                        obf[:, b, :].rearrange("p (f h) -> p f h", h=4),
                                        pb[:, 0:256].rearrange("p (f h) -> p f h", h=4),
                                        rc[:].unsqueeze(1)
                                        .broadcast_to((128, 64, 4)),
                                        Alu.mult)
                                else:
                                    obb = ew.tile([128, 256], bf16, tag="obb")
                                    nc.vector.tensor_tensor(
                                        obb[:].rearrange("p (f h) -> p f h", h=4),
                                        pb[:, 0:256].rearrange("p (f h) -> p f h", h=4),
                                        rc[:].unsqueeze(1)
                                        .broadcast_to((128, 64, 4)),
                                        Alu.mult)
                                    # -------- per-block fusion --------
                                    ct = ew.tile([128, 4, 128], bf16, tag="ct")
                                    for k, src in enumerate(
                                            (obf[:, b, 0:128], obf[:, b, 128:256],
                                             obb[:, 0:128], obb[:, 128:256])):
                                        pt = ppb.tile([128, 128], bf16, tag="pt", bufs=1)
                                        nc.tensor.transpose(pt[:], src, idb_sb[:])
                                        nc.scalar.activation(ct[:, k, :], pt[:], Act.Copy)
                                    pf = ppf.tile([128, 512], f32, tag="pf",
                                                  name=f"pf_{b}", bufs=1)
                                    for k in range(4):
                                        nc.tensor.matmul(pf[:], ct[:, k, :],
                                                         wf_sb[:, k, :],
                                                         start=(k == 0), stop=(k == 3))
                                    nc.scalar.activation(fused[:, b, :], pf[:], Act.Copy)
                                    sq = ew.tile([128, 512], bf16, tag="sq")
                                    nc.vector.tensor_tensor(sq[:], fused[:, b, :],
                                                            fused[:, b, :], Alu.mult)
                                    nc.tensor.matmul(stat1[:], ones_col[:],
                                                     fused[:, b, :],
                                                     start=(b == 0), stop=(b == nb - 1))
                                    nc.tensor.matmul(stat2[:], ones_col[:], sq[:],
                                                     start=(b == 0), stop=(b == nb - 1))

                # ---------------- BN tail ----------------
                with tc.tile_pool(name="tail", bufs=1) as tl:
                    stat_sa = tl.tile([1, 512], f32)
                    nc.vector.tensor_copy(stat_sa[:], stat1[:])
                    stat_sbb = tl.tile([1, 512], f32)
                    nc.vector.tensor_copy(stat_sbb[:], stat2[:])
                    nc.sync.dma_start(ccin[0:1, :], stat_sa[:])
                    nc.sync.dma_start(ccin[1:2, :], stat_sbb[:])
                    sga = tl.tile([1, 512], f32)
                    sgb = tl.tile([1, 512], f32)
                    if USE_CC:
                        nc.gpsimd.collective_compute(
                            "AllReduce", Alu.add,
                            replica_groups=[list(range(NCORES))],
                            ins=[ccin[:]], outs=[ccout[:]])
                        nc.sync.dma_start(sga[:], ccout[0:1, :])
                        nc.sync.dma_start(sgb[:], ccout[1:2, :])
                    else:
                        nc.sync.dma_start(sga[:], ccin[0:1, :])
                        nc.sync.dma_start(sgb[:], ccin[1:2, :])
                    m = tl.tile([1, 512], f32)
                    nc.vector.tensor_scalar_mul(m[:], sga[:], 1.0 / n_nodes)
                    e2 = tl.tile([1, 512], f32)
                    nc.vector.tensor_scalar_mul(e2[:], sgb[:], 1.0 / n_nodes)
                    var = tl.tile([1, 512], f32)
                    nc.vector.scalar_tensor_tensor(
                        var[:], m[:], 1.0, m[:], Alu.mult, Alu.mult)
                    nc.vector.tensor_tensor(var[:], e2[:], var[:], Alu.subtract)
                    nc.vector.tensor_scalar_add(var[:], var[:], BN_EPS)
                    sd = tl.tile([1, 512], f32)
                    nc.scalar.activation(sd[:], var[:], Act.Sqrt)
                    rs = tl.tile([1, 512], f32)
                    nc.vector.reciprocal(rs[:], sd[:])
                    A = tl.tile([1, 512], f32)
                    nc.vector.tensor_tensor(A[:], rs[:], gam_sb[:], Alu.mult)
                    mA = tl.tile([1, 512], f32)
                    nc.vector.tensor_tensor(mA[:], m[:], A[:], Alu.mult)
                    B = tl.tile([1, 512], f32)
                    nc.vector.tensor_tensor(B[:], bet_sb[:], mA[:], Alu.subtract)
                    pA = ppf.tile([128, 512], f32, tag="pA", bufs=1)
                    nc.tensor.matmul(pA[:], ones1[:], A[:])
                    pB = ppf.tile([128, 512], f32, tag="pB", bufs=1)
                    nc.tensor.matmul(pB[:], ones1[:], B[:])
                    pA_sb = tl.tile([128, 512], f32)
                    nc.scalar.activation(pA_sb[:], pA[:], Act.Copy)
                    pB_sb = tl.tile([128, 512], f32)
                    nc.scalar.activation(pB_sb[:], pB[:], Act.Copy)
                    with tc.tile_pool(name="norm", bufs=3) as nw:
                        for b in range(nb):
                            t1 = nw.tile([128, 512], f32, tag="t1")
                            eng = nc.vector if b % 2 == 0 else nc.gpsimd
                            eng.tensor_tensor(t1[:], fused[:, b, :], pA_sb[:], Alu.mult)
                            eng.tensor_tensor(t1[:], t1[:], pB_sb[:], Alu.add)
                            eng.tensor_scalar_max(t1[:], t1[:], 0.0)
                            rows = min(128, npc - b * 128)
                            nc.sync.dma_start(out_d[b * 128:b * 128 + rows, :],
                                              t1[0:rows, :])
    nc.compile()
    return nc


def kernel(**inputs):
    x = np.asarray(inputs["x"], dtype=np.float32)
    ei = np.asarray(inputs["edge_index"])
    n_nodes, D = x.shape
    npc, nb, half, trows_lo, trows_hi = _derive(n_nodes)
    n_all = ((n_nodes + 127) // 128) * 128
    npad = nb * 128

    def g(name):
        return np.asarray(inputs[name], dtype=np.float32)

    W_f, W_b = g("W_f"), g("W_b")
    asf, adf = g("att_src_f"), g("att_dst_f")
    asb, adb = g("att_src_b"), g("att_dst_b")
    W_fuse = g("W_fuse")
    gamma, beta = g("bn_gamma"), g("bn_beta")

    # (c,h)-interleaved weight layout: col c*4+h <- W[:, h, c]
    wall = np.zeros((512, 528), dtype=np.float32)
    wall[:, 0:256] = W_f.transpose(0, 2, 1).reshape(512, 256)
    wall[:, 256:260] = np.einsum("dhc,hc->dh", W_f, asf)
    wall[:, 260:516] = W_b.transpose(0, 2, 1).reshape(512, 256)
    wall[:, 516:520] = np.einsum("dhc,hc->dh", W_b, asb)
    wall[:, 520:524] = np.einsum("dhc,hc->dh", W_f, adf)
    wall[:, 524:528] = np.einsum("dhc,hc->dh", W_b, adb)

    # W_fuse rows permuted to the (c,h)-interleaved combined layout
    wfp = np.zeros_like(W_fuse)
    hc = np.arange(256)
    h_, c_ = hc // 64, hc % 64
    wfp[c_ * 4 + h_, :] = W_fuse[hc, :]
    wfp[256 + c_ * 4 + h_, :] = W_fuse[256 + hc, :]

    xT = np.zeros((512, n_all), dtype=ml_dtypes.bfloat16)
    drow = np.zeros((1, TSTRIDE), dtype=ml_dtypes.bfloat16)
    drow[0, 256:260] = DUMMY_AS
    iota = np.broadcast_to(np.arange(128), (128, 128)).astype(ml_dtypes.bfloat16)
    ident = np.eye(128, dtype=np.float32)
    gb = np.stack([gamma, beta]).astype(np.float32)

    src0, dst0 = ei[0].astype(np.int64), ei[1].astype(np.int64)
    newpos = _balance(src0, dst0, n_nodes)
    order = np.argsort(newpos)          # order[new_id] = old_id
    src, dst = newpos[src0], newpos[dst0]
    clo_f, chi_f, g1f, g2f, dpf = _prep_edges(src, dst, n_nodes)
    clo_b, chi_b, g1b, g2b, dpb = _prep_edges(dst, src, n_nodes)

    xT[:, :n_nodes] = x.T[:, order]
    nc = _build_program(n_nodes, clo_f, chi_f, clo_b, chi_b)

    in_maps = []
    for c in range(NCORES):
        xTo = np.zeros((512, npad), dtype=ml_dtypes.bfloat16)
        xTo[:, :npc] = x.T[:, order[c * npc:(c + 1) * npc]]
        in_maps.append({
            "xT": xT, "xTo": xTo,
            "wall": wall.astype(ml_dtypes.bfloat16),
            "wfuse": wfp.astype(ml_dtypes.bfloat16),
            "drow": drow, "iota": iota, "ident": ident,
            "identb": ident.astype(ml_dtypes.bfloat16), "gb": gb,
            "g1f": g1f[c], "g2f": g2f[c], "dpf": dpf[c].astype(np.float32),
            "g1b": g1b[c], "g2b": g2b[c], "dpb": dpb[c].astype(np.float32),
        })
    kernel.last_nc = nc
    res = run_bass_kernel_spmd(nc, in_maps, list(range(NCORES)))
    out = np.concatenate([np.asarray(res.results[c]["out"]) for c in range(NCORES)], axis=0)
    out_final = np.empty((n_nodes, 512), dtype=np.float32)
    out_final[order] = out[:n_nodes]
    return out_final


if __name__ == "__main__":
    pass


# revision 19
# speedup vs baseline: 2.1543x; 1.0582x over previous
"""Bidirectional GATConv + fusion + BatchNorm + ReLU on 8 Trainium2 cores.

v2 design (cost-model driven):
  - DMA descriptor-time is the hard floor (all transfers serialize on the
    DMA_ENGINES device). Tables shrunk to 520B rows (260 bf16 cols,
    (c,h)-interleaved), a_d table to 32B rows, xT loaded with 2KB
    descriptors, no `combined` DRAM round-trip.
  - Edge inner loop: one batched DVE tensor_tensor for the alpha-weighting
    (ext broadcast rides a non-last stride-0 dim so 2x bf16 packing holds),
    one tensor_scalar one-hot per chunk, exp writes straight into the
    message tile's denominator columns so one 260-row matmul per chunk
    scatters messages + denominators.
  - Fusion per dst-block via PE transposes (no DRAM transpose), BN stats
    via ones-matmul PSUM accumulation, 4KB AllReduce, normalize+ReLU
    split across DVE/Pool.
Biases provably cancel through BatchNorm and are dropped.
"""
import sys

sys.path.insert(0, "/opt/trn_rl_repo")

import numpy as np
import ml_dtypes

import concourse.bass as bass
import concourse.bacc as bacc
import concourse.mybir as mybir
from concourse import tile
from concourse import library_config
from concourse.bass_utils import run_bass_kernel_spmd

bf16 = mybir.dt.bfloat16
f32 = mybir.dt.float32
i16 = mybir.dt.int16
Alu = mybir.AluOpType
Act = mybir.ActivationFunctionType

NCORES = 8
USE_CC = __import__("os").environ.get("NO_CC", "0") != "1"
KB = 2          # dst blocks per gather supergroup
SLOT = 7        # identity-scatter slot chunks per (block, half)
SPILL = 2       # one-hot spill chunks per (block, half)
NEG_SLOPE = 0.2
BN_EPS = 1e-5
DUMMY_AS = -60.0
TROW = 260      # written cols: 256 (c,h)-interleaved h + 4 a_s
TSTRIDE = 384   # physical table row stride (768B, gather elem must be 256B-mult)
XW = 1024       # xT load column batch (2KB descriptors)


def _derive(n_nodes):
    npc = n_nodes // NCORES
    nb = (npc + 127) // 128
    half = ((n_nodes // 2) // 128) * 128
    trows_lo = half + 64            # dummy row at half+16
    trows_hi = (n_nodes - half) + NCORES * 16 + 64
    return npc, nb, half, trows_lo, trows_hi


def _pack_idx(arr):
    """int16 [n] (n%16==0) -> [128, n/16] wrapped in 16 partitions, replicated per Q7 core."""
    a = arr.reshape(-1, 16).T
    return np.tile(a, (8, 1)).astype(np.int16)


def _prep_edges(gidx, anode, n_nodes):
    """Host edge partitioning, identity-slot + spill layout.

    Per (core, block, half): SLOT chunks hold the k-th edge of each dst at
    partition = dstpos (identity scatter, local a_d); SPILL chunks hold
    overflow edges in packed order with a dstpos stream (one-hot scatter,
    a_d via 256B gather). Chunk count per (block, half) is SLOT+SPILL.
    """
    npc, nb, half, _, _ = _derive(n_nodes)
    n_all = ((n_nodes + 127) // 128) * 128
    dlo, dhi = half + 16, (n_all - half) + 16
    core = anode // npc
    local = anode - core * npc
    block = local // 128
    dstpos = local % 128
    hi = (gidx >= half).astype(np.int64)
    lidx = gidx - hi * half
    cc = SLOT + SPILL

    # rank of each edge within its (core, block, half, dst) group
    grp = ((core * nb + block) * 2 + hi) * 128 + dstpos
    oe = np.argsort(grp, kind="stable")
    gs = grp[oe]
    counts = np.bincount(gs, minlength=NCORES * nb * 2 * 128)
    starts = np.concatenate(([0], np.cumsum(counts)))[:-1]
    rank = np.arange(len(gs)) - starts[gs]

    # slot tables [core, block, half, k, dstpos]
    g1slot = np.full((NCORES, nb, 2, SLOT, 128), -1, dtype=np.int64)
    sel = rank < SLOT
    e = oe[sel]
    g1slot[core[e], block[e], hi[e], rank[sel], dstpos[e]] = lidx[e]
    dummy = np.where(np.arange(2) == 0, dlo, dhi)  # per half
    for h in range(2):
        sl = g1slot[:, :, h]
        sl[sl < 0] = dummy[h]

    # spill: packed per (core, block, half)
    se = oe[~sel]
    sgrp = (core[se] * nb + block[se]) * 2 + hi[se]
    so = np.argsort(sgrp, kind="stable")
    se2 = se[so]
    sg2 = sgrp[so]
    scnt = np.bincount(sg2, minlength=NCORES * nb * 2)
    assert scnt.max() <= SPILL * 128, f"spill overflow {scnt.max()}"
    sstarts = np.concatenate(([0], np.cumsum(scnt)))[:-1]
    q = np.arange(len(se2)) - sstarts[sg2]
    g1sp = np.empty((NCORES, nb, 2, SPILL, 128), dtype=np.int64)
    g1sp[:, :, 0] = dlo
    g1sp[:, :, 1] = dhi
    g2sp = np.zeros((NCORES, nb, 2, SPILL, 128), dtype=np.int64)
    dpsp = np.full((NCORES, nb, 2, SPILL, 128), 200.0, dtype=np.float32)
    c_, b_, h_ = sg2 // (nb * 2), (sg2 // 2) % nb, sg2 % 2
    g1sp[c_, b_, h_, q // 128, q % 128] = lidx[se2]
    g2sp[c_, b_, h_, q // 128, q % 128] = block[se2] * 128 + dstpos[se2]
    dpsp[c_, b_, h_, q // 128, q % 128] = dstpos[se2]
    # pad g2 entries point at the block's first row
    bb = np.broadcast_to(np.arange(nb)[None, :, None, None, None] * 128,
                         g2sp.shape).copy()
    padm = dpsp == 200.0
    g2sp[padm] = bb[padm]

    # per-block chunk sequence: [slot 0..SLOT-1, spill 0..SPILL-1]
    g1all = np.concatenate([g1slot, g1sp], axis=3)   # [C, nb, 2, cc, 128]

    # g1 gather streams: per supergroup: [lo blocks chunk-major | hi ...]
    g1_streams, g2_streams = [], []
    for c in range(NCORES):
        p1, p2 = [], []
        for bs in range(0, nb, KB):
            be = min(bs + KB, nb)
            p1 += [_pack_idx(g1all[c, bs:be, 0].ravel()),
                   _pack_idx(g1all[c, bs:be, 1].ravel())]
            # at-gather idxs: [block][half][spill k][128]
            p2.append(_pack_idx(g2sp[c, bs:be].ravel()))
        g1_streams.append(np.concatenate(p1, axis=1))
        g2_streams.append(np.concatenate(p2, axis=1))

    # dp stream for spill chunks: [128, nb*2*SPILL], col = (b*2+h)*SPILL+k
    dp_stream = dpsp.transpose(0, 4, 1, 2, 3).reshape(
        NCORES, 128, nb * 2 * SPILL)
    return np.stack(g1_streams), np.stack(g2_streams), np.ascontiguousarray(dp_stream)


def _balance(src, dst, n_nodes):
    """Assign nodes to (core, block) bins so every bin's 4 incident-edge
    counts (f_lo, f_hi, b_lo, b_hi) are <= 1024 (8 chunks of 128), keeping
    each node's half-class so neighbor half membership stays fixed.
    Returns newpos[old_id] -> new_id."""
    import time as _time
    npc, nb, half, _, _ = _derive(n_nodes)
    nbins = NCORES * nb
    cap = np.full(nbins, 128, dtype=np.int64)
    for c in range(NCORES):
        cap[c * nb + nb - 1] = npc - (nb - 1) * 128
    is_lo_src = src < half
    is_lo_dst = dst < half
    dd = np.zeros((n_nodes, 4), dtype=np.int64)
    np.add.at(dd[:, 0], dst[is_lo_src], 1)
    np.add.at(dd[:, 1], dst[~is_lo_src], 1)
    np.add.at(dd[:, 2], src[is_lo_dst], 1)
    np.add.at(dd[:, 3], src[~is_lo_dst], 1)
    d = np.maximum(dd - SLOT, 0)   # spill edges per (dir, half)
    cap_lo = np.zeros(nbins, dtype=np.int64)
    for c in range(NCORES):
        for b in range(nb):
            i = c * nb + b
            start = c * npc + b * 128
            cap_lo[i] = max(0, min(start + cap[i], half) - start)
    cap_hi = cap - cap_lo

    rng = np.random.default_rng(0)
    sums = np.zeros((nbins, 4), dtype=np.int64)
    slots_lo, slots_hi = cap_lo.copy(), cap_hi.copy()
    assign = np.empty(n_nodes, dtype=np.int64)
    order_nodes = np.argsort(-d.sum(1), kind="stable")
    for n in order_nodes:
        slots = slots_lo if n < half else slots_hi
        feas = slots > 0
        cand = sums[feas] + d[n]
        score = cand.max(1) / (SPILL * 128.0) - 1e-6 * slots[feas]
        j = np.flatnonzero(feas)[np.argmin(score)]
        assign[n] = j
        sums[j] += d[n]
        slots[j] -= 1
    # swap-repair toward all bins <= 1024 on all 4 dims
    half_class = np.arange(n_nodes) < half
    members = [np.flatnonzero(assign == i).tolist() for i in range(nbins)]
    t0 = _time.time()
    CAPS = SPILL * 128
    for it in range(200000):
        over = np.argwhere(sums > CAPS)
        if len(over) == 0 or _time.time() - t0 > 90:
            break
        A, dim = over[rng.integers(len(over))]
        n1 = members[A][rng.integers(len(members[A]))]
        h1 = half_class[n1]
        for _ in range(60):
            B = int(rng.integers(nbins))
            if B == A or not members[B]:
                continue
            k2 = int(rng.integers(len(members[B])))
            n2 = members[B][k2]
            if half_class[n2] != h1:
                continue
            dA = sums[A] - d[n1] + d[n2]
            dB = sums[B] - d[n2] + d[n1]
            if (dA <= np.maximum(sums[A], CAPS)).all() and dA[dim] < sums[A][dim] \
               and (dB <= CAPS).all():
                sums[A] = dA
                sums[B] = dB
                members[A].remove(n1)
                members[B][k2] = n1
                members[A].append(n2)
                assign[n1], assign[n2] = B, A
                break
    # slots within bins: lo-class nodes take the bin's lo prefix
    newpos = np.empty(n_nodes, dtype=np.int64)
    for i in range(nbins):
        c, b = divmod(i, nb)
        start = c * npc + b * 128
        mem = np.array(members[i], dtype=np.int64)
        lo_m = mem[half_class[mem]]
        hi_m = mem[~half_class[mem]]
        newpos[lo_m] = start + np.arange(len(lo_m))
        newpos[hi_m] = start + cap_lo[i] + np.arange(len(hi_m))
    return newpos

def _build_program(n_nodes):
    npc, nb, half, trows_lo, trows_hi = _derive(n_nodes)
    npad = nb * 128
    n_all = ((n_nodes + 127) // 128) * 128
    npb = n_all // 128                     # projection node blocks
    dummy_lo = half + 16
    dummy_hi = (n_all - half) + 16
    hblocks = half // 128

    nc = bacc.Bacc(None, target_bir_lowering=False)
    inp = lambda name, shape, dt: nc.declare_dram_parameter(name, shape, dt, isOutput=False)
    xT = inp("xT", [512, n_all], bf16)
    xTo = inp("xTo", [512, npad], bf16)
    wall = inp("wall", [512, 528], bf16)
    wfuse = inp("wfuse", [512, 512], bf16)
    drow = inp("drow", [1, TSTRIDE], bf16)
    iota_in = inp("iota", [128, 128], bf16)
    ident = inp("ident", [128, 128], f32)
    identb = inp("identb", [128, 128], bf16)
    gb = inp("gb", [2, 512], f32)
    cc = SLOT + SPILL
    streams = {}
    for d in "fb":
        streams["g1" + d] = inp("g1" + d, [128, nb * 2 * cc * 8], i16)
        streams["g2" + d] = inp("g2" + d, [128, nb * 2 * SPILL * 8], i16)
        streams["dp" + d] = inp("dp" + d, [128, nb * 2 * SPILL], f32)
    out_d = nc.declare_dram_parameter("out", [npc, 512], f32, isOutput=True)

    tabs = {d: [nc.dram_tensor(f"tab{d}{h}", [tr, TSTRIDE], bf16)
                for h, tr in (("lo", trows_lo), ("hi", trows_hi))] for d in "fb"}
    adtab = nc.dram_tensor("adtab", [npad, 128], bf16)
    ccin = nc.dram_tensor("ccin", [2, 512], f32)
    ccout = nc.dram_tensor("ccout", [2, 512], f32, addr_space="Shared")

    with tile.TileContext(nc) as tc:
        with tc.tile_pool(name="const", bufs=1) as cpool:
            nc.gpsimd.load_library(library_config.mlp)
            wall_sb = cpool.tile([128, 4, 528], bf16)
            for k in range(4):
                nc.sync.dma_start(wall_sb[:, k, :], wall[k * 128:(k + 1) * 128, :])
            wf_sb = cpool.tile([128, 4, 512], bf16)
            for k in range(4):
                nc.sync.dma_start(wf_sb[:, k, :], wfuse[k * 128:(k + 1) * 128, :])
            iota_sb = cpool.tile([128, 128], bf16)
            nc.sync.dma_start(iota_sb[:], iota_in[:])
            idb_sb = cpool.tile([128, 128], bf16)
            nc.sync.dma_start(idb_sb[:], identb[:])
            gam_sb = cpool.tile([1, 512], f32)
            nc.sync.dma_start(gam_sb[:], gb[0:1, :])
            bet_sb = cpool.tile([1, 512], f32)
            nc.sync.dma_start(bet_sb[:], gb[1:2, :])
            ones_col = cpool.tile([128, 1], bf16)
            nc.vector.memset(ones_col[:], 1.0)
            ones1 = cpool.tile([1, 128], f32)
            nc.vector.memset(ones1[:], 1.0)
            # zero-fill table tail rows, then dummy rows
            zt = cpool.tile([128, TSTRIDE], bf16)
            nc.gpsimd.memset(zt[:], 0.0)
            for d in "fb":
                r = half
                while r < trows_lo:
                    n = min(128, trows_lo - r)
                    nc.sync.dma_start(tabs[d][0][r:r + n, :], zt[0:n, :])
                    r += n
                r = n_all - half
                while r < trows_hi:
                    n = min(128, trows_hi - r)
                    nc.sync.dma_start(tabs[d][1][r:r + n, :], zt[0:n, :])
                    r += n
                nc.sync.dma_start(tabs[d][0][dummy_lo:dummy_lo + 1, :], drow[:])
                nc.sync.dma_start(tabs[d][1][dummy_hi:dummy_hi + 1, :], drow[:])

            # persistent slabs
            obf = cpool.tile([128, nb, 256], bf16)
            fused = cpool.tile([128, nb, 512], bf16)
            ad_slab = cpool.tile([128, nb, 8], bf16)

            # ---------------- projection (both dirs, one xT pass) ----------------
            with (tc.tile_pool(name="proj", bufs=2) as pj,
                  tc.tile_pool(name="pspj", bufs=2, space="PSUM") as pp):
                ngrp = (npb + 7) // 8
                for g in range(ngrp):
                    b0 = g * 8
                    nblk = min(8, npb - b0)
                    w = nblk * 128
                    xt = pj.tile([128, 4, XW], bf16, tag="xt")
                    nc.sync.dma_start(
                        xt[:, :, 0:w],
                        xT[:, b0 * 128:b0 * 128 + w]
                        .rearrange("(k p) n -> p k n", p=128))
                    stg = pj.tile([128, 8, 520], bf16, tag="stg")
                    for j in range(nblk):
                        psf = pp.tile([128, 260], f32, tag="psf", name=f"psf_{g}_{j}")
                        psb = pp.tile([128, 260], f32, tag="psb", name=f"psb_{g}_{j}")
                        for k in range(4):
                            nc.tensor.matmul(psf[:], xt[:, k, j * 128:(j + 1) * 128],
                                             wall_sb[:, k, 0:260],
                                             start=(k == 0), stop=(k == 3))
                        for k in range(4):
                            nc.tensor.matmul(psb[:], xt[:, k, j * 128:(j + 1) * 128],
                                             wall_sb[:, k, 260:520],
                                             start=(k == 0), stop=(k == 3))
                        nc.scalar.activation(stg[:, j, 0:260], psf[:], Act.Copy)
                        nc.vector.tensor_copy(stg[:, j, 260:520], psb[:])
                    # batched table writes (handle lo/hi straddle)
                    for d, c0 in (("f", 0), ("b", 260)):
                        j = 0
                        while j < nblk:
                            blk = b0 + j
                            if blk < hblocks:
                                nmax = min(nblk - j, hblocks - blk)
                                dst = tabs[d][0][blk * 128:(blk + nmax) * 128, 0:TROW]
                            else:
                                nmax = nblk - j
                                r0 = (blk - hblocks) * 128
                                dst = tabs[d][1][r0:r0 + nmax * 128, 0:TROW]
                            nc.sync.dma_start(
                                dst.rearrange("(j p) c -> p j c", p=128),
                                stg[:, j:j + nmax, c0:c0 + TROW])
                            j += nmax
                # local a_d table (own shard via xTo)
                ngo = (nb * 128 + XW - 1) // XW
                for g in range(ngo):
                    b0 = g * 8
                    nblk = min(8, nb - b0)
                    w = nblk * 128
                    xo = pj.tile([128, 4, XW], bf16, tag="xt")
                    nc.sync.dma_start(
                        xo[:, :, 0:w],
                        xTo[:, b0 * 128:b0 * 128 + w]
                        .rearrange("(k p) n -> p k n", p=128))
                    for j in range(nblk):
                        pa = pp.tile([128, 8], f32, tag="pa", bufs=1)
                        for k in range(4):
                            nc.tensor.matmul(pa[:], xo[:, k, j * 128:(j + 1) * 128],
                                             wall_sb[:, k, 520:528],
                                             start=(k == 0), stop=(k == 3))
                        nc.vector.tensor_copy(ad_slab[:, b0 + j, :], pa[:])
                nc.sync.dma_start(
                    adtab.rearrange("(b p) c -> p b c", p=128)[:, :, 0:8], ad_slab[:])

            # ---------------- edge passes + fusion ----------------
            # PSUM pools that live across both edge passes
            with (tc.tile_pool(name="psed", bufs=2, space="PSUM") as ppb,
                  tc.tile_pool(name="psfu", bufs=2, space="PSUM") as ppf,
                  tc.tile_pool(name="psst", bufs=1, space="PSUM") as pps,
                  tc.tile_pool(name="edges", bufs=1) as es):
                stat1 = pps.tile([1, 512], f32, name="stat1")
                stat2 = pps.tile([1, 512], f32, name="stat2")
                NCC = SLOT + SPILL
                for d, adofs in (("f", 0), ("b", 4)):
                    dp_sb = es.tile([128, nb * 2 * SPILL], f32, tag="dp" + d,
                                    name="dp" + d)
                    nc.sync.dma_start(dp_sb[:], streams["dp" + d][:])
                    with (tc.tile_pool(name="est" + d, bufs=2) as est,
                          tc.tile_pool(name="ew" + d, bufs=2) as ew):
                        c1 = c2 = 0
                        for bs in range(0, nb, KB):
                            kbs = min(KB, nb - bs)
                            nsec = kbs * NCC * 128
                            span1 = 2 * nsec // 16
                            g1s = est.tile([128, KB * 2 * NCC * 8], i16, tag="g1s")
                            nc.scalar.dma_start(g1s[:, 0:span1],
                                                streams["g1" + d][:, c1:c1 + span1])
                            c1 += span1
                            nsp = kbs * 2 * SPILL * 128
                            span2 = nsp // 16
                            g2s = est.tile([128, KB * 2 * SPILL * 8], i16, tag="g2s")
                            nc.scalar.dma_start(g2s[:, 0:span2],
                                                streams["g2" + d][:, c2:c2 + span2])
                            c2 += span2
                            gtl = est.tile([128, KB * NCC, TSTRIDE], bf16, tag="gtl")
                            nc.gpsimd.dma_gather(
                                gtl[:, 0:kbs * NCC, :], tabs[d][0][:],
                                g1s[:, 0:nsec // 16], num_idxs=nsec,
                                num_idxs_reg=nsec, elem_size=TSTRIDE,
                                single_packet=False)
                            gth = est.tile([128, KB * NCC, TSTRIDE], bf16, tag="gth")
                            nc.gpsimd.dma_gather(
                                gth[:, 0:kbs * NCC, :], tabs[d][1][:],
                                g1s[:, nsec // 16:span1], num_idxs=nsec,
                                num_idxs_reg=nsec, elem_size=TSTRIDE,
                                single_packet=False)
                            at = est.tile([128, KB * 2 * SPILL, 128], bf16, tag="at")
                            nc.gpsimd.dma_gather(
                                at[:, 0:kbs * 2 * SPILL, :], adtab[:],
                                g2s[:, 0:span2], num_idxs=nsp,
                                num_idxs_reg=nsp, elem_size=128,
                                single_packet=False)
                            for j in range(kbs):
                                b = bs + j
                                pb = ppb.tile([128, 260], f32, tag="pb",
                                              name=f"pb{d}_{b}")
                                first = True
                                for half_i, gt in ((0, gtl), (1, gth)):
                                    sl = slice(j * NCC, (j + 1) * NCC)
                                    atof = (j * 2 + half_i) * SPILL
                                    dpof = (b * 2 + half_i) * SPILL
                                    et = ew.tile([128, NCC, 4], bf16, tag="et")
                                    nc.vector.tensor_tensor(
                                        et[:, 0:SLOT, :],
                                        gt[:, j * NCC:j * NCC + SLOT, 256:260],
                                        ad_slab[:, b, adofs:adofs + 4].unsqueeze(1)
                                        .broadcast_to((128, SLOT, 4)), Alu.add)
                                    nc.vector.tensor_tensor(
                                        et[:, SLOT:NCC, :],
                                        gt[:, j * NCC + SLOT:(j + 1) * NCC, 256:260],
                                        at[:, atof:atof + SPILL, adofs:adofs + 4],
                                        Alu.add)
                                    lt = ew.tile([128, NCC * 4], f32, tag="lt")
                                    nc.vector.scalar_tensor_tensor(
                                        lt[:, 0:NCC * 4],
                                        et[:].rearrange("p c h -> p (c h)"),
                                        NEG_SLOPE,
                                        et[:].rearrange("p c h -> p (c h)"),
                                        Alu.mult, Alu.max)
                                    mt = ew.tile([128, NCC, 260], bf16, tag="mt")
                                    nc.scalar.activation(
                                        mt[:, :, 256:260],
                                        lt[:].rearrange("p (c h) -> p c h", h=4),
                                        Act.Exp)
                                    nc.vector.tensor_tensor(
                                        mt[:, :, 0:256].rearrange(
                                            "p c (f h) -> p c f h", h=4),
                                        gt[:, sl, 0:256].rearrange(
                                            "p c (f h) -> p c f h", h=4),
                                        mt[:, :, 256:260].unsqueeze(2)
                                        .broadcast_to((128, NCC, 64, 4)),
                                        Alu.mult)
                                    st = ew.tile([128, SPILL, 128], bf16, tag="st")
                                    for k in range(SPILL):
                                        nc.vector.tensor_scalar(
                                            st[:, k, :], iota_sb[:],
                                            dp_sb[:, dpof + k:dpof + k + 1], None,
                                            op0=Alu.is_equal)
                                    for k in range(NCC):
                                        last = half_i == 1 and k == NCC - 1
                                        lhsT = idb_sb[:] if k < SLOT else st[:, k - SLOT, :]
                                        nc.tensor.matmul(pb[:], lhsT, mt[:, k, :],
                                                         start=first, stop=last)
                                        first = False
                                dn = ew.tile([128, 4], f32, tag="dn")
                                nc.vector.tensor_scalar_add(dn[:], pb[:, 256:260], 1e-16)
                                rc = ew.tile([128, 4], f32, tag="rc")
                                nc.vector.reciprocal(rc[:], dn[:])
                                if d == "f":
                                    nc.vector.tensor_tensor(
                                        obf[:, b, :].rearrange("p (f h) -> p f h", h=4),
                                        pb[:, 0:256].rearrange("p (f h) -> p f h", h=4),
                                        rc[:].unsqueeze(1)
                                        .broadcast_to((128, 64, 4)),
                                        Alu.mult)
                                else:
                                    obb = ew.tile([128, 256], bf16, tag="obb")
                                    nc.vector.tensor_tensor(
                                        obb[:].rearrange("p (f h) -> p f h", h=4),
                                        pb[:, 0:256].rearrange("p (f h) -> p f h", h=4),
                                        rc[:].unsqueeze(1)
                                        .broadcast_to((128, 64, 4)),
                                        Alu.mult)
                                    # -------- per-block fusion --------
                                    ct = ew.tile([128, 4, 128], bf16, tag="ct")
                                    for k, srcv in enumerate(
                                            (obf[:, b, 0:128], obf[:, b, 128:256],
                                             obb[:, 0:128], obb[:, 128:256])):
                                        pt = ppb.tile([128, 128], bf16, tag="pt", bufs=1)
                                        nc.tensor.transpose(pt[:], srcv, idb_sb[:])
                                        nc.scalar.activation(ct[:, k, :], pt[:], Act.Copy)
                                    pf = ppf.tile([128, 512], f32, tag="pf",
                                                  name=f"pf_{b}", bufs=1)
                                    for k in range(4):
                                        nc.tensor.matmul(pf[:], ct[:, k, :],
                                                         wf_sb[:, k, :],
                                                         start=(k == 0), stop=(k == 3))
                                    nc.scalar.activation(fused[:, b, :], pf[:], Act.Copy)
                                    sq = ew.tile([128, 512], bf16, tag="sq")
                                    nc.vector.tensor_tensor(sq[:], fused[:, b, :],
                                                            fused[:, b, :], Alu.mult)
                                    nc.tensor.matmul(stat1[:], ones_col[:],
                                                     fused[:, b, :],
                                                     start=(b == 0), stop=(b == nb - 1))
                                    nc.tensor.matmul(stat2[:], ones_col[:], sq[:],
                                                     start=(b == 0), stop=(b == nb - 1))

                # ---------------- BN tail ----------------
                with tc.tile_pool(name="tail", bufs=1) as tl:
                    stat_sa = tl.tile([1, 512], f32)
                    nc.vector.tensor_copy(stat_sa[:], stat1[:])
                    stat_sbb = tl.tile([1, 512], f32)
                    nc.vector.tensor_copy(stat_sbb[:], stat2[:])
                    nc.sync.dma_start(ccin[0:1, :], stat_sa[:])
                    nc.sync.dma_start(ccin[1:2, :], stat_sbb[:])
                    sga = tl.tile([1, 512], f32)
                    sgb = tl.tile([1, 512], f32)
                    if USE_CC:
                        nc.gpsimd.collective_compute(
                            "AllReduce", Alu.add,
                            replica_groups=[list(range(NCORES))],
                            ins=[ccin[:]], outs=[ccout[:]])
                        nc.sync.dma_start(sga[:], ccout[0:1, :])
                        nc.sync.dma_start(sgb[:], ccout[1:2, :])
                    else:
                        nc.sync.dma_start(sga[:], ccin[0:1, :])
                        nc.sync.dma_start(sgb[:], ccin[1:2, :])
                    m = tl.tile([1, 512], f32)
                    nc.vector.tensor_scalar_mul(m[:], sga[:], 1.0 / n_nodes)
                    e2 = tl.tile([1, 512], f32)
                    nc.vector.tensor_scalar_mul(e2[:], sgb[:], 1.0 / n_nodes)
                    var = tl.tile([1, 512], f32)
                    nc.vector.scalar_tensor_tensor(
                        var[:], m[:], 1.0, m[:], Alu.mult, Alu.mult)
                    nc.vector.tensor_tensor(var[:], e2[:], var[:], Alu.subtract)
                    nc.vector.tensor_scalar_add(var[:], var[:], BN_EPS)
                    sd = tl.tile([1, 512], f32)
                    nc.scalar.activation(sd[:], var[:], Act.Sqrt)
                    rs = tl.tile([1, 512], f32)
                    nc.vector.reciprocal(rs[:], sd[:])
                    A = tl.tile([1, 512], f32)
                    nc.vector.tensor_tensor(A[:], rs[:], gam_sb[:], Alu.mult)
                    mA = tl.tile([1, 512], f32)
                    nc.vector.tensor_tensor(mA[:], m[:], A[:], Alu.mult)
                    B = tl.tile([1, 512], f32)
                    nc.vector.tensor_tensor(B[:], bet_sb[:], mA[:], Alu.subtract)
                    pA = ppf.tile([128, 512], f32, tag="pA", bufs=1)
                    nc.tensor.matmul(pA[:], ones1[:], A[:])
                    pB = ppf.tile([128, 512], f32, tag="pB", bufs=1)
                    nc.tensor.matmul(pB[:], ones1[:], B[:])
                    pA_sb = tl.tile([128, 512], f32)
                    nc.scalar.activation(pA_sb[:], pA[:], Act.Copy)
                    pB_sb = tl.tile([128, 512], f32)
                    nc.scalar.activation(pB_sb[:], pB[:], Act.Copy)
                    with tc.tile_pool(name="norm", bufs=3) as nw:
                        for b in range(nb):
                            t1 = nw.tile([128, 512], f32, tag="t1")
                            eng = nc.vector if b % 2 == 0 else nc.gpsimd
                            eng.tensor_tensor(t1[:], fused[:, b, :], pA_sb[:], Alu.mult)
                            eng.tensor_tensor(t1[:], t1[:], pB_sb[:], Alu.add)
                            eng.tensor_scalar_max(t1[:], t1[:], 0.0)
                            rows = min(128, npc - b * 128)
                            nc.sync.dma_start(out_d[b * 128:b * 128 + rows, :],
                                              t1[0:rows, :])
    nc.compile()
    return nc


def kernel(**inputs):
    x = np.asarray(inputs["x"], dtype=np.float32)
    ei = np.asarray(inputs["edge_index"])
    n_nodes, D = x.shape
    npc, nb, half, trows_lo, trows_hi = _derive(n_nodes)
    n_all = ((n_nodes + 127) // 128) * 128
    npad = nb * 128

    def g(name):
        return np.asarray(inputs[name], dtype=np.float32)

    W_f, W_b = g("W_f"), g("W_b")
    asf, adf = g("att_src_f"), g("att_dst_f")
    asb, adb = g("att_src_b"), g("att_dst_b")
    W_fuse = g("W_fuse")
    gamma, beta = g("bn_gamma"), g("bn_beta")

    # (c,h)-interleaved weight layout: col c*4+h <- W[:, h, c]
    wall = np.zeros((512, 528), dtype=np.float32)
    wall[:, 0:256] = W_f.transpose(0, 2, 1).reshape(512, 256)
    wall[:, 256:260] = np.einsum("dhc,hc->dh", W_f, asf)
    wall[:, 260:516] = W_b.transpose(0, 2, 1).reshape(512, 256)
    wall[:, 516:520] = np.einsum("dhc,hc->dh", W_b, asb)
    wall[:, 520:524] = np.einsum("dhc,hc->dh", W_f, adf)
    wall[:, 524:528] = np.einsum("dhc,hc->dh", W_b, adb)

    # W_fuse rows permuted to the (c,h)-interleaved combined layout
    wfp = np.zeros_like(W_fuse)
    hc = np.arange(256)
    h_, c_ = hc // 64, hc % 64
    wfp[c_ * 4 + h_, :] = W_fuse[hc, :]
    wfp[256 + c_ * 4 + h_, :] = W_fuse[256 + hc, :]

    xT = np.zeros((512, n_all), dtype=ml_dtypes.bfloat16)
    drow = np.zeros((1, TSTRIDE), dtype=ml_dtypes.bfloat16)
    drow[0, 256:260] = DUMMY_AS
    iota = np.broadcast_to(np.arange(128), (128, 128)).astype(ml_dtypes.bfloat16)
    ident = np.eye(128, dtype=np.float32)
    gb = np.stack([gamma, beta]).astype(np.float32)

    src0, dst0 = ei[0].astype(np.int64), ei[1].astype(np.int64)
    newpos = _balance(src0, dst0, n_nodes)
    order = np.argsort(newpos)          # order[new_id] = old_id
    src, dst = newpos[src0], newpos[dst0]
    g1f, g2f, dpf = _prep_edges(src, dst, n_nodes)
    g1b, g2b, dpb = _prep_edges(dst, src, n_nodes)

    xT[:, :n_nodes] = x.T[:, order]
    nc = _build_program(n_nodes)

    in_maps = []
    for c in range(NCORES):
        xTo = np.zeros((512, npad), dtype=ml_dtypes.bfloat16)
        xTo[:, :npc] = x.T[:, order[c * npc:(c + 1) * npc]]
        in_maps.append({
            "xT": xT, "xTo": xTo,
            "wall": wall.astype(ml_dtypes.bfloat16),
            "wfuse": wfp.astype(ml_dtypes.bfloat16),
            "drow": drow, "iota": iota, "ident": ident,
            "identb": ident.astype(ml_dtypes.bfloat16), "gb": gb,
            "g1f": g1f[c], "g2f": g2f[c], "dpf": dpf[c],
            "g1b": g1b[c], "g2b": g2b[c], "dpb": dpb[c],
        })
    kernel.last_nc = nc
    res = run_bass_kernel_spmd(nc, in_maps, list(range(NCORES)))
    out = np.concatenate([np.asarray(res.results[c]["out"]) for c in range(NCORES)], axis=0)
    out_final = np.empty((n_nodes, 512), dtype=np.float32)
    out_final[order] = out[:n_nodes]
    return out_final


if __name__ == "__main__":
    pass


# revision 24
# speedup vs baseline: 2.1981x; 1.0203x over previous
"""Bidirectional GATConv + fusion + BatchNorm + ReLU on 8 Trainium2 cores.

v2 design (cost-model driven):
  - DMA descriptor-time is the hard floor (all transfers serialize on the
    DMA_ENGINES device). Tables shrunk to 520B rows (260 bf16 cols,
    (c,h)-interleaved), a_d table to 32B rows, xT loaded with 2KB
    descriptors, no `combined` DRAM round-trip.
  - Edge inner loop: one batched DVE tensor_tensor for the alpha-weighting
    (ext broadcast rides a non-last stride-0 dim so 2x bf16 packing holds),
    one tensor_scalar one-hot per chunk, exp writes straight into the
    message tile's denominator columns so one 260-row matmul per chunk
    scatters messages + denominators.
  - Fusion per dst-block via PE transposes (no DRAM transpose), BN stats
    via ones-matmul PSUM accumulation, 4KB AllReduce, normalize+ReLU
    split across DVE/Pool.
Biases provably cancel through BatchNorm and are dropped.
"""
import sys

sys.path.insert(0, "/opt/trn_rl_repo")

import numpy as np
import ml_dtypes

import concourse.bass as bass
import concourse.bacc as bacc
import concourse.mybir as mybir
from concourse import tile
from concourse import library_config
from concourse.bass_utils import run_bass_kernel_spmd

bf16 = mybir.dt.bfloat16
f32 = mybir.dt.float32
i16 = mybir.dt.int16
Alu = mybir.AluOpType
Act = mybir.ActivationFunctionType

NCORES = 8
USE_CC = __import__("os").environ.get("NO_CC", "0") != "1"
KB = 2          # dst blocks per gather supergroup
SLOT = 7        # identity-scatter slot chunks per (block, half)
SPILL = 2       # one-hot spill chunks per (block, half)
NEG_SLOPE = 0.2
BN_EPS = 1e-5
DUMMY_AS = -60.0
TROW = 260      # written cols: 256 (c,h)-interleaved h + 4 a_s
TSTRIDE = 384   # physical table row stride (768B, gather elem must be 256B-mult)
XW = 1024       # xT load column batch (2KB descriptors)


def _derive(n_nodes):
    npc = n_nodes // NCORES
    nb = (npc + 127) // 128
    half = ((n_nodes // 2) // 128) * 128
    trows_lo = half + 64            # dummy row at half+16
    trows_hi = (n_nodes - half) + NCORES * 16 + 64
    return npc, nb, half, trows_lo, trows_hi


def _pack_idx(arr):
    """int16 [n] (n%16==0) -> [128, n/16] wrapped in 16 partitions, replicated per Q7 core."""
    a = arr.reshape(-1, 16).T
    return np.tile(a, (8, 1)).astype(np.int16)


def _prep_edges(gidx, anode, n_nodes):
    """Host edge partitioning, identity-slot + spill layout.

    Per (core, block, half): SLOT chunks hold the k-th edge of each dst at
    partition = dstpos (identity scatter, local a_d); SPILL chunks hold
    overflow edges in packed order with a dstpos stream (one-hot scatter,
    a_d via 256B gather). Chunk count per (block, half) is SLOT+SPILL.
    """
    npc, nb, half, _, _ = _derive(n_nodes)
    n_all = ((n_nodes + 127) // 128) * 128
    dlo, dhi = half + 16, (n_all - half) + 16
    core = anode // npc
    local = anode - core * npc
    block = local // 128
    dstpos = local % 128
    hi = (gidx >= half).astype(np.int64)
    lidx = gidx - hi * half
    cc = SLOT + SPILL

    # rank of each edge within its (core, block, half, dst) group
    grp = ((core * nb + block) * 2 + hi) * 128 + dstpos
    oe = np.argsort(grp, kind="stable")
    gs = grp[oe]
    counts = np.bincount(gs, minlength=NCORES * nb * 2 * 128)
    starts = np.concatenate(([0], np.cumsum(counts)))[:-1]
    rank = np.arange(len(gs)) - starts[gs]

    # slot tables [core, block, half, k, dstpos]
    g1slot = np.full((NCORES, nb, 2, SLOT, 128), -1, dtype=np.int64)
    sel = rank < SLOT
    e = oe[sel]
    g1slot[core[e], block[e], hi[e], rank[sel], dstpos[e]] = lidx[e]
    dummy = np.where(np.arange(2) == 0, dlo, dhi)  # per half
    for h in range(2):
        sl = g1slot[:, :, h]
        sl[sl < 0] = dummy[h]

    # spill: packed per (core, block, half)
    se = oe[~sel]
    sgrp = (core[se] * nb + block[se]) * 2 + hi[se]
    so = np.argsort(sgrp, kind="stable")
    se2 = se[so]
    sg2 = sgrp[so]
    scnt = np.bincount(sg2, minlength=NCORES * nb * 2)
    assert scnt.max() <= SPILL * 128, f"spill overflow {scnt.max()}"
    sstarts = np.concatenate(([0], np.cumsum(scnt)))[:-1]
    q = np.arange(len(se2)) - sstarts[sg2]
    g1sp = np.empty((NCORES, nb, 2, SPILL, 128), dtype=np.int64)
    g1sp[:, :, 0] = dlo
    g1sp[:, :, 1] = dhi
    g2sp = np.zeros((NCORES, nb, 2, SPILL, 128), dtype=np.int64)
    dpsp = np.full((NCORES, nb, 2, SPILL, 128), 200.0, dtype=np.float32)
    c_, b_, h_ = sg2 // (nb * 2), (sg2 // 2) % nb, sg2 % 2
    g1sp[c_, b_, h_, q // 128, q % 128] = lidx[se2]
    g2sp[c_, b_, h_, q // 128, q % 128] = block[se2] * 128 + dstpos[se2]
    dpsp[c_, b_, h_, q // 128, q % 128] = dstpos[se2]
    # pad g2 entries point at the block's first row
    bb = np.broadcast_to(np.arange(nb)[None, :, None, None, None] * 128,
                         g2sp.shape).copy()
    padm = dpsp == 200.0
    g2sp[padm] = bb[padm]

    # per-block chunk sequence: [slot 0..SLOT-1, spill 0..SPILL-1]
    g1all = np.concatenate([g1slot, g1sp], axis=3)   # [C, nb, 2, cc, 128]

    # g1 gather streams: per supergroup: [lo blocks chunk-major | hi ...]
    g1_streams, g2_streams = [], []
    for c in range(NCORES):
        p1, p2 = [], []
        for bs in range(0, nb, KB):
            be = min(bs + KB, nb)
            p1 += [_pack_idx(g1all[c, bs:be, 0].ravel()),
                   _pack_idx(g1all[c, bs:be, 1].ravel())]
            # at-gather idxs: [block][half][spill k][128]
            p2.append(_pack_idx(g2sp[c, bs:be].ravel()))
        g1_streams.append(np.concatenate(p1, axis=1))
        g2_streams.append(np.concatenate(p2, axis=1))

    # dp stream for spill chunks: [128, nb*2*SPILL], col = (b*2+h)*SPILL+k
    dp_stream = dpsp.transpose(0, 4, 1, 2, 3).reshape(
        NCORES, 128, nb * 2 * SPILL)
    return np.stack(g1_streams), np.stack(g2_streams), np.ascontiguousarray(dp_stream)


def _balance(src, dst, n_nodes):
    """Assign nodes to (core, block) bins so every bin's 4 incident-edge
    counts (f_lo, f_hi, b_lo, b_hi) are <= 1024 (8 chunks of 128), keeping
    each node's half-class so neighbor half membership stays fixed.
    Returns newpos[old_id] -> new_id."""
    import time as _time
    npc, nb, half, _, _ = _derive(n_nodes)
    nbins = NCORES * nb
    cap = np.full(nbins, 128, dtype=np.int64)
    for c in range(NCORES):
        cap[c * nb + nb - 1] = npc - (nb - 1) * 128
    is_lo_src = src < half
    is_lo_dst = dst < half
    dd = np.zeros((n_nodes, 4), dtype=np.int64)
    np.add.at(dd[:, 0], dst[is_lo_src], 1)
    np.add.at(dd[:, 1], dst[~is_lo_src], 1)
    np.add.at(dd[:, 2], src[is_lo_dst], 1)
    np.add.at(dd[:, 3], src[~is_lo_dst], 1)
    d = np.maximum(dd - SLOT, 0)   # spill edges per (dir, half)
    cap_lo = np.zeros(nbins, dtype=np.int64)
    for c in range(NCORES):
        for b in range(nb):
            i = c * nb + b
            start = c * npc + b * 128
            cap_lo[i] = max(0, min(start + cap[i], half) - start)
    cap_hi = cap - cap_lo

    rng = np.random.default_rng(0)
    sums = np.zeros((nbins, 4), dtype=np.int64)
    slots_lo, slots_hi = cap_lo.copy(), cap_hi.copy()
    assign = np.empty(n_nodes, dtype=np.int64)
    order_nodes = np.argsort(-d.sum(1), kind="stable")
    for n in order_nodes:
        slots = slots_lo if n < half else slots_hi
        feas = slots > 0
        cand = sums[feas] + d[n]
        score = cand.max(1) / (SPILL * 128.0) - 1e-6 * slots[feas]
        j = np.flatnonzero(feas)[np.argmin(score)]
        assign[n] = j
        sums[j] += d[n]
        slots[j] -= 1
    # swap-repair toward all bins <= 1024 on all 4 dims
    half_class = np.arange(n_nodes) < half
    members = [np.flatnonzero(assign == i).tolist() for i in range(nbins)]
    t0 = _time.time()
    CAPS = SPILL * 128
    for it in range(200000):
        over = np.argwhere(sums > CAPS)
        if len(over) == 0 or _time.time() - t0 > 90:
            break
        A, dim = over[rng.integers(len(over))]
        n1 = members[A][rng.integers(len(members[A]))]
        h1 = half_class[n1]
        for _ in range(60):
            B = int(rng.integers(nbins))
            if B == A or not members[B]:
                continue
            k2 = int(rng.integers(len(members[B])))
            n2 = members[B][k2]
            if half_class[n2] != h1:
                continue
            dA = sums[A] - d[n1] + d[n2]
            dB = sums[B] - d[n2] + d[n1]
            if (dA <= np.maximum(sums[A], CAPS)).all() and dA[dim] < sums[A][dim] \
               and (dB <= CAPS).all():
                sums[A] = dA
                sums[B] = dB
                members[A].remove(n1)
                members[B][k2] = n1
                members[A].append(n2)
                assign[n1], assign[n2] = B, A
                break
    # slots within bins: lo-class nodes take the bin's lo prefix
    newpos = np.empty(n_nodes, dtype=np.int64)
    for i in range(nbins):
        c, b = divmod(i, nb)
        start = c * npc + b * 128
        mem = np.array(members[i], dtype=np.int64)
        lo_m = mem[half_class[mem]]
        hi_m = mem[~half_class[mem]]
        newpos[lo_m] = start + np.arange(len(lo_m))
        newpos[hi_m] = start + cap_lo[i] + np.arange(len(hi_m))
    return newpos

def _build_program(n_nodes):
    npc, nb, half, trows_lo, trows_hi = _derive(n_nodes)
    npad = nb * 128
    n_all = ((n_nodes + 127) // 128) * 128
    npb = n_all // 128                     # projection node blocks
    dummy_lo = half + 16
    dummy_hi = (n_all - half) + 16
    hblocks = half // 128

    nc = bacc.Bacc(None, target_bir_lowering=False)
    inp = lambda name, shape, dt: nc.declare_dram_parameter(name, shape, dt, isOutput=False)
    xT = inp("xT", [512, n_all], bf16)
    xTo = inp("xTo", [512, npad], bf16)
    wall = inp("wall", [512, 528], bf16)
    wfuse = inp("wfuse", [512, 512], bf16)
    drow = inp("drow", [1, TSTRIDE], bf16)
    iota_in = inp("iota", [128, 128], bf16)
    ident = inp("ident", [128, 128], f32)
    identb = inp("identb", [128, 128], bf16)
    gb = inp("gb", [2, 512], f32)
    cc = SLOT + SPILL
    streams = {}
    for d in "fb":
        streams["g1" + d] = inp("g1" + d, [128, nb * 2 * cc * 8], i16)
        streams["g2" + d] = inp("g2" + d, [128, nb * 2 * SPILL * 8], i16)
        streams["dp" + d] = inp("dp" + d, [128, nb * 2 * SPILL], f32)
    out_d = nc.declare_dram_parameter("out", [npc, 512], f32, isOutput=True)

    tabs = {d: [nc.dram_tensor(f"tab{d}{h}", [tr, TSTRIDE], bf16)
                for h, tr in (("lo", trows_lo), ("hi", trows_hi))] for d in "fb"}
    adtab = nc.dram_tensor("adtab", [npad, 128], bf16)
    ccin = nc.dram_tensor("ccin", [2, 512], f32)
    ccout = nc.dram_tensor("ccout", [16, 512], f32, addr_space="Shared")

    with tile.TileContext(nc) as tc:
        with tc.tile_pool(name="const", bufs=1) as cpool:
            nc.gpsimd.load_library(library_config.mlp)
            wall_sb = cpool.tile([128, 4, 528], bf16)
            for k in range(4):
                nc.sync.dma_start(wall_sb[:, k, :], wall[k * 128:(k + 1) * 128, :])
            wf_sb = cpool.tile([128, 4, 512], bf16)
            for k in range(4):
                nc.sync.dma_start(wf_sb[:, k, :], wfuse[k * 128:(k + 1) * 128, :])
            iota_sb = cpool.tile([128, 128], bf16)
            nc.sync.dma_start(iota_sb[:], iota_in[:])
            idb_sb = cpool.tile([128, 128], bf16)
            nc.sync.dma_start(idb_sb[:], identb[:])
            gam_sb = cpool.tile([1, 512], f32)
            nc.sync.dma_start(gam_sb[:], gb[0:1, :])
            bet_sb = cpool.tile([1, 512], f32)
            nc.sync.dma_start(bet_sb[:], gb[1:2, :])
            ones_col = cpool.tile([128, 1], bf16)
            nc.vector.memset(ones_col[:], 1.0)
            ones1 = cpool.tile([1, 128], f32)
            nc.vector.memset(ones1[:], 1.0)
            # zero-fill table tail rows, then dummy rows
            zt = cpool.tile([128, TSTRIDE], bf16)
            nc.gpsimd.memset(zt[:], 0.0)
            for d in "fb":
                r = half
                while r < trows_lo:
                    n = min(128, trows_lo - r)
                    nc.sync.dma_start(tabs[d][0][r:r + n, :], zt[0:n, :])
                    r += n
                r = n_all - half
                while r < trows_hi:
                    n = min(128, trows_hi - r)
                    nc.sync.dma_start(tabs[d][1][r:r + n, :], zt[0:n, :])
                    r += n
                nc.sync.dma_start(tabs[d][0][dummy_lo:dummy_lo + 1, :], drow[:])
                nc.sync.dma_start(tabs[d][1][dummy_hi:dummy_hi + 1, :], drow[:])

            # persistent slabs
            obf = cpool.tile([128, nb, 256], bf16)
            fused = cpool.tile([128, nb, 512], bf16)
            ad_slab = cpool.tile([128, nb, 8], bf16)

            # ---------------- projection (both dirs, one xT pass) ----------------
            with (tc.tile_pool(name="proj", bufs=2) as pj,
                  tc.tile_pool(name="pspj", bufs=2, space="PSUM") as pp):
                ngrp = (npb + 7) // 8
                for g in range(ngrp):
                    b0 = g * 8
                    nblk = min(8, npb - b0)
                    w = nblk * 128
                    xt = pj.tile([128, 4, XW], bf16, tag="xt")
                    nc.sync.dma_start(
                        xt[:, :, 0:w],
                        xT[:, b0 * 128:b0 * 128 + w]
                        .rearrange("(k p) n -> p k n", p=128))
                    stg = pj.tile([128, 8, 520], bf16, tag="stg")
                    for j in range(nblk):
                        psf = pp.tile([128, 260], f32, tag="psf", name=f"psf_{g}_{j}")
                        psb = pp.tile([128, 260], f32, tag="psb", name=f"psb_{g}_{j}")
                        for k in range(4):
                            nc.tensor.matmul(psf[:], xt[:, k, j * 128:(j + 1) * 128],
                                             wall_sb[:, k, 0:260],
                                             start=(k == 0), stop=(k == 3))
                        for k in range(4):
                            nc.tensor.matmul(psb[:], xt[:, k, j * 128:(j + 1) * 128],
                                             wall_sb[:, k, 260:520],
                                             start=(k == 0), stop=(k == 3))
                        nc.scalar.activation(stg[:, j, 0:260], psf[:], Act.Copy)
                        nc.vector.tensor_copy(stg[:, j, 260:520], psb[:])
                    # batched table writes (handle lo/hi straddle)
                    for d, c0 in (("f", 0), ("b", 260)):
                        j = 0
                        while j < nblk:
                            blk = b0 + j
                            if blk < hblocks:
                                nmax = min(nblk - j, hblocks - blk)
                                dst = tabs[d][0][blk * 128:(blk + nmax) * 128, 0:TROW]
                            else:
                                nmax = nblk - j
                                r0 = (blk - hblocks) * 128
                                dst = tabs[d][1][r0:r0 + nmax * 128, 0:TROW]
                            nc.sync.dma_start(
                                dst.rearrange("(j p) c -> p j c", p=128),
                                stg[:, j:j + nmax, c0:c0 + TROW])
                            j += nmax
                # local a_d table (own shard via xTo)
                ngo = (nb * 128 + XW - 1) // XW
                for g in range(ngo):
                    b0 = g * 8
                    nblk = min(8, nb - b0)
                    w = nblk * 128
                    xo = pj.tile([128, 4, XW], bf16, tag="xt")
                    nc.sync.dma_start(
                        xo[:, :, 0:w],
                        xTo[:, b0 * 128:b0 * 128 + w]
                        .rearrange("(k p) n -> p k n", p=128))
                    for j in range(nblk):
                        pa = pp.tile([128, 8], f32, tag="pa", bufs=1)
                        for k in range(4):
                            nc.tensor.matmul(pa[:], xo[:, k, j * 128:(j + 1) * 128],
                                             wall_sb[:, k, 520:528],
                                             start=(k == 0), stop=(k == 3))
                        nc.vector.tensor_copy(ad_slab[:, b0 + j, :], pa[:])
                nc.sync.dma_start(
                    adtab.rearrange("(b p) c -> p b c", p=128)[:, :, 0:8], ad_slab[:])

            # ---------------- edge passes + fusion ----------------
            # PSUM pools that live across both edge passes
            with (tc.tile_pool(name="psed", bufs=2, space="PSUM") as ppb,
                  tc.tile_pool(name="psfu", bufs=2, space="PSUM") as ppf,
                  tc.tile_pool(name="psst", bufs=1, space="PSUM") as pps,
                  tc.tile_pool(name="edges", bufs=1) as es):
                stat1 = pps.tile([1, 512], f32, name="stat1")
                stat2 = pps.tile([1, 512], f32, name="stat2")
                NCC = SLOT + SPILL
                for d, adofs in (("f", 0), ("b", 4)):
                    dp_sb = es.tile([128, nb * 2 * SPILL], f32, tag="dp" + d,
                                    name="dp" + d)
                    nc.sync.dma_start(dp_sb[:], streams["dp" + d][:])
                    with (tc.tile_pool(name="est" + d, bufs=3) as est,
                          tc.tile_pool(name="ew" + d, bufs=2) as ew):
                        c1 = c2 = 0
                        for bs in range(0, nb, KB):
                            kbs = min(KB, nb - bs)
                            nsec = kbs * NCC * 128
                            span1 = 2 * nsec // 16
                            g1s = est.tile([128, KB * 2 * NCC * 8], i16, tag="g1s")
                            nc.scalar.dma_start(g1s[:, 0:span1],
                                                streams["g1" + d][:, c1:c1 + span1])
                            c1 += span1
                            nsp = kbs * 2 * SPILL * 128
                            span2 = nsp // 16
                            g2s = est.tile([128, KB * 2 * SPILL * 8], i16, tag="g2s")
                            nc.scalar.dma_start(g2s[:, 0:span2],
                                                streams["g2" + d][:, c2:c2 + span2])
                            c2 += span2
                            gtl = est.tile([128, KB * NCC, TSTRIDE], bf16, tag="gtl")
                            nc.gpsimd.dma_gather(
                                gtl[:, 0:kbs * NCC, :], tabs[d][0][:],
                                g1s[:, 0:nsec // 16], num_idxs=nsec,
                                num_idxs_reg=nsec, elem_size=TSTRIDE,
                                single_packet=False)
                            gth = est.tile([128, KB * NCC, TSTRIDE], bf16, tag="gth")
                            nc.gpsimd.dma_gather(
                                gth[:, 0:kbs * NCC, :], tabs[d][1][:],
                                g1s[:, nsec // 16:span1], num_idxs=nsec,
                                num_idxs_reg=nsec, elem_size=TSTRIDE,
                                single_packet=False)
                            at = est.tile([128, KB * 2 * SPILL, 128], bf16, tag="at")
                            nc.gpsimd.dma_gather(
                                at[:, 0:kbs * 2 * SPILL, :], adtab[:],
                                g2s[:, 0:span2], num_idxs=nsp,
                                num_idxs_reg=nsp, elem_size=128,
                                single_packet=False)
                            for j in range(kbs):
                                b = bs + j
                                pb = ppb.tile([128, 260], f32, tag="pb",
                                              name=f"pb{d}_{b}")
                                first = True
                                for half_i, gt in ((0, gtl), (1, gth)):
                                    sl = slice(j * NCC, (j + 1) * NCC)
                                    atof = (j * 2 + half_i) * SPILL
                                    dpof = (b * 2 + half_i) * SPILL
                                    et = ew.tile([128, NCC, 4], bf16, tag="et")
                                    nc.vector.tensor_tensor(
                                        et[:, 0:SLOT, :],
                                        gt[:, j * NCC:j * NCC + SLOT, 256:260],
                                        ad_slab[:, b, adofs:adofs + 4].unsqueeze(1)
                                        .broadcast_to((128, SLOT, 4)), Alu.add)
                                    nc.vector.tensor_tensor(
                                        et[:, SLOT:NCC, :],
                                        gt[:, j * NCC + SLOT:(j + 1) * NCC, 256:260],
                                        at[:, atof:atof + SPILL, adofs:adofs + 4],
                                        Alu.add)
                                    lt = ew.tile([128, NCC * 4], f32, tag="lt")
                                    nc.vector.scalar_tensor_tensor(
                                        lt[:, 0:NCC * 4],
                                        et[:].rearrange("p c h -> p (c h)"),
                                        NEG_SLOPE,
                                        et[:].rearrange("p c h -> p (c h)"),
                                        Alu.mult, Alu.max)
                                    mt = ew.tile([128, NCC, 260], bf16, tag="mt")
                                    nc.scalar.activation(
                                        mt[:, :, 256:260],
                                        lt[:].rearrange("p (c h) -> p c h", h=4),
                                        Act.Exp)
                                    nc.vector.tensor_tensor(
                                        mt[:, :, 0:256].rearrange(
                                            "p c (f h) -> p c f h", h=4),
                                        gt[:, sl, 0:256].rearrange(
                                            "p c (f h) -> p c f h", h=4),
                                        mt[:, :, 256:260].unsqueeze(2)
                                        .broadcast_to((128, NCC, 64, 4)),
                                        Alu.mult)
                                    st = ew.tile([128, SPILL, 128], bf16, tag="st")
                                    for k in range(SPILL):
                                        nc.vector.tensor_scalar(
                                            st[:, k, :], iota_sb[:],
                                            dp_sb[:, dpof + k:dpof + k + 1], None,
                                            op0=Alu.is_equal)
                                    for k in range(NCC):
                                        last = half_i == 1 and k == NCC - 1
                                        lhsT = idb_sb[:] if k < SLOT else st[:, k - SLOT, :]
                                        nc.tensor.matmul(pb[:], lhsT, mt[:, k, :],
                                                         start=first, stop=last)
                                        first = False
                                dn = ew.tile([128, 4], f32, tag="dn")
                                nc.vector.tensor_scalar_add(dn[:], pb[:, 256:260], 1e-16)
                                rc = ew.tile([128, 4], f32, tag="rc")
                                nc.vector.reciprocal(rc[:], dn[:])
                                if d == "f":
                                    nc.vector.tensor_tensor(
                                        obf[:, b, :].rearrange("p (f h) -> p f h", h=4),
                                        pb[:, 0:256].rearrange("p (f h) -> p f h", h=4),
                                        rc[:].unsqueeze(1)
                                        .broadcast_to((128, 64, 4)),
                                        Alu.mult)
                                else:
                                    obb = ew.tile([128, 256], bf16, tag="obb")
                                    nc.vector.tensor_tensor(
                                        obb[:].rearrange("p (f h) -> p f h", h=4),
                                        pb[:, 0:256].rearrange("p (f h) -> p f h", h=4),
                                        rc[:].unsqueeze(1)
                                        .broadcast_to((128, 64, 4)),
                                        Alu.mult)
                                    # -------- per-block fusion --------
                                    ct = ew.tile([128, 4, 128], bf16, tag="ct")
                                    for k, srcv in enumerate(
                                            (obf[:, b, 0:128], obf[:, b, 128:256],
                                             obb[:, 0:128], obb[:, 128:256])):
                                        pt = ppb.tile([128, 128], bf16, tag="pt", bufs=1)
                                        nc.tensor.transpose(pt[:], srcv, idb_sb[:])
                                        nc.scalar.activation(ct[:, k, :], pt[:], Act.Copy)
                                    pf = ppf.tile([128, 512], f32, tag="pf",
                                                  name=f"pf_{b}", bufs=1)
                                    for k in range(4):
                                        nc.tensor.matmul(pf[:], ct[:, k, :],
                                                         wf_sb[:, k, :],
                                                         start=(k == 0), stop=(k == 3))
                                    nc.scalar.activation(fused[:, b, :], pf[:], Act.Copy)
                                    sq = ew.tile([128, 512], bf16, tag="sq")
                                    nc.vector.tensor_tensor(sq[:], fused[:, b, :],
                                                            fused[:, b, :], Alu.mult)
                                    nc.tensor.matmul(stat1[:], ones_col[:],
                                                     fused[:, b, :],
                                                     start=(b == 0), stop=(b == nb - 1))
                                    nc.tensor.matmul(stat2[:], ones_col[:], sq[:],
                                                     start=(b == 0), stop=(b == nb - 1))

                # ---------------- BN tail ----------------
                with tc.tile_pool(name="tail", bufs=1) as tl:
                    stat_sa = tl.tile([1, 512], f32)
                    nc.vector.tensor_copy(stat_sa[:], stat1[:])
                    stat_sbb = tl.tile([1, 512], f32)
                    nc.vector.tensor_copy(stat_sbb[:], stat2[:])
                    nc.sync.dma_start(ccin[0:1, :], stat_sa[:])
                    nc.sync.dma_start(ccin[1:2, :], stat_sbb[:])
                    sga = tl.tile([1, 512], f32)
                    sgb = tl.tile([1, 512], f32)
                    if USE_CC:
                        nc.gpsimd.collective_compute(
                            "AllGather", Alu.bypass,
                            replica_groups=[list(range(NCORES))],
                            ins=[ccin[:]], outs=[ccout[:]])
                        allst = tl.tile([8, 1024], f32)
                        nc.sync.dma_start(
                            allst[:], ccout.rearrange("(r a) c -> r (a c)", a=2))
                        ones8 = tl.tile([8, 1], f32)
                        nc.vector.memset(ones8[:], 1.0)
                        pra = ppf.tile([1, 512], f32, tag="pA", bufs=1, name="pra")
                        nc.tensor.matmul(pra[:], ones8[:], allst[:, 0:512])
                        prb = ppf.tile([1, 512], f32, tag="pB", bufs=1, name="prb")
                        nc.tensor.matmul(prb[:], ones8[:], allst[:, 512:1024])
                        nc.vector.tensor_copy(sga[:], pra[:])
                        nc.vector.tensor_copy(sgb[:], prb[:])
                    else:
                        nc.sync.dma_start(sga[:], ccin[0:1, :])
                        nc.sync.dma_start(sgb[:], ccin[1:2, :])
                    m = tl.tile([1, 512], f32)
                    nc.vector.tensor_scalar_mul(m[:], sga[:], 1.0 / n_nodes)
                    e2 = tl.tile([1, 512], f32)
                    nc.vector.tensor_scalar_mul(e2[:], sgb[:], 1.0 / n_nodes)
                    var = tl.tile([1, 512], f32)
                    nc.vector.scalar_tensor_tensor(
                        var[:], m[:], 1.0, m[:], Alu.mult, Alu.mult)
                    nc.vector.tensor_tensor(var[:], e2[:], var[:], Alu.subtract)
                    nc.vector.tensor_scalar_add(var[:], var[:], BN_EPS)
                    sd = tl.tile([1, 512], f32)
                    nc.scalar.activation(sd[:], var[:], Act.Sqrt)
                    rs = tl.tile([1, 512], f32)
                    nc.vector.reciprocal(rs[:], sd[:])
                    A = tl.tile([1, 512], f32)
                    nc.vector.tensor_tensor(A[:], rs[:], gam_sb[:], Alu.mult)
                    mA = tl.tile([1, 512], f32)
                    nc.vector.tensor_tensor(mA[:], m[:], A[:], Alu.mult)
                    B = tl.tile([1, 512], f32)
                    nc.vector.tensor_tensor(B[:], bet_sb[:], mA[:], Alu.subtract)
                    pA = ppf.tile([128, 512], f32, tag="pA", bufs=1)
                    nc.tensor.matmul(pA[:], ones1[:], A[:])
                    pB = ppf.tile([128, 512], f32, tag="pB", bufs=1)
                    nc.tensor.matmul(pB[:], ones1[:], B[:])
                    pA_sb = tl.tile([128, 512], f32)
                    nc.scalar.activation(pA_sb[:], pA[:], Act.Copy)
                    pB_sb = tl.tile([128, 512], f32)
                    nc.scalar.activation(pB_sb[:], pB[:], Act.Copy)
                    with tc.tile_pool(name="norm", bufs=3) as nw:
                        for b in range(nb):
                            t1 = nw.tile([128, 512], f32, tag="t1")
                            eng = nc.vector if b % 2 == 0 else nc.gpsimd
                            eng.tensor_tensor(t1[:], fused[:, b, :], pA_sb[:], Alu.mult)
                            eng.tensor_tensor(t1[:], t1[:], pB_sb[:], Alu.add)
                            t2 = nw.tile([128, 512], f32, tag="t2")
                            nc.scalar.activation(t2[:], t1[:], Act.Relu)
                            rows = min(128, npc - b * 128)
                            nc.sync.dma_start(out_d[b * 128:b * 128 + rows, :],
                                              t2[0:rows, :])
    nc.compile()
    return nc


def kernel(**inputs):
    x = np.asarray(inputs["x"], dtype=np.float32)
    ei = np.asarray(inputs["edge_index"])
    n_nodes, D = x.shape
    npc, nb, half, trows_lo, trows_hi = _derive(n_nodes)
    n_all = ((n_nodes + 127) // 128) * 128
    npad = nb * 128

    def g(name):
        return np.asarray(inputs[name], dtype=np.float32)

    W_f, W_b = g("W_f"), g("W_b")
    asf, adf = g("att_src_f"), g("att_dst_f")
    asb, adb = g("att_src_b"), g("att_dst_b")
    W_fuse = g("W_fuse")
    gamma, beta = g("bn_gamma"), g("bn_beta")

    # (c,h)-interleaved weight layout: col c*4+h <- W[:, h, c]
    wall = np.zeros((512, 528), dtype=np.float32)
    wall[:, 0:256] = W_f.transpose(0, 2, 1).reshape(512, 256)
    wall[:, 256:260] = np.einsum("dhc,hc->dh", W_f, asf)
    wall[:, 260:516] = W_b.transpose(0, 2, 1).reshape(512, 256)
    wall[:, 516:520] = np.einsum("dhc,hc->dh", W_b, asb)
    wall[:, 520:524] = np.einsum("dhc,hc->dh", W_f, adf)
    wall[:, 524:528] = np.einsum("dhc,hc->dh", W_b, adb)

    # W_fuse rows permuted to the (c,h)-interleaved combined layout
    wfp = np.zeros_like(W_fuse)
    hc = np.arange(256)
    h_, c_ = hc // 64, hc % 64
    wfp[c_ * 4 + h_, :] = W_fuse[hc, :]
    wfp[256 + c_ * 4 + h_, :] = W_fuse[256 + hc, :]

    xT = np.zeros((512, n_all), dtype=ml_dtypes.bfloat16)
    drow = np.zeros((1, TSTRIDE), dtype=ml_dtypes.bfloat16)
    drow[0, 256:260] = DUMMY_AS
    iota = np.broadcast_to(np.arange(128), (128, 128)).astype(ml_dtypes.bfloat16)
    ident = np.eye(128, dtype=np.float32)
    gb = np.stack([gamma, beta]).astype(np.float32)

    src0, dst0 = ei[0].astype(np.int64), ei[1].astype(np.int64)
    newpos = _balance(src0, dst0, n_nodes)
    order = np.argsort(newpos)          # order[new_id] = old_id
    src, dst = newpos[src0], newpos[dst0]
    g1f, g2f, dpf = _prep_edges(src, dst, n_nodes)
    g1b, g2b, dpb = _prep_edges(dst, src, n_nodes)

    xT[:, :n_nodes] = x.T[:, order]
    nc = _build_program(n_nodes)

    in_maps = []
    for c in range(NCORES):
        xTo = np.zeros((512, npad), dtype=ml_dtypes.bfloat16)
        xTo[:, :npc] = x.T[:, order[c * npc:(c + 1) * npc]]
        in_maps.append({
            "xT": xT, "xTo": xTo,
            "wall": wall.astype(ml_dtypes.bfloat16),
            "wfuse": wfp.astype(ml_dtypes.bfloat16),
            "drow": drow, "iota": iota, "ident": ident,
            "identb": ident.astype(ml_dtypes.bfloat16), "gb": gb,
            "g1f": g1f[c], "g2f": g2f[c], "dpf": dpf[c],
            "g1b": g1b[c], "g2b": g2b[c], "dpb": dpb[c],
        })
    kernel.last_nc = nc
    res = run_bass_kernel_spmd(nc, in_maps, list(range(NCORES)))
    out = np.concatenate([np.asarray(res.results[c]["out"]) for c in range(NCORES)], axis=0)
    out_final = np.empty((n_nodes, 512), dtype=np.float32)
    out_final[order] = out[:n_nodes]
    return out_final


if __name__ == "__main__":
    pass
